# revision 1
# baseline (speedup 1.0000x reference)
"""Bass/Trainium2 kernel for nn_BysMamba (bidirectional + stacked Mamba LM).

Sharding: tensor-parallel over d_inner ED=944 across 8 cores (118 channels
each). Every core keeps the full residual stream h (DIM=472 x B*L tokens,
fp32 master in DRAM), computes its channel shard of each Mamba block
(in_proj, causal conv as diagonal matmuls on the tensor engine, selective
scan via DVE tensor_tensor_scan with fp32 internal state, gating, out_proj
partial); partial x_proj / out_proj contractions are summed with AllReduce.
"""
import sys
sys.path.insert(0, '/opt/trn_rl_repo')

import numpy as np
import ml_dtypes

import concourse.bass as bass
from concourse import bacc
import concourse.mybir as mybir
import concourse.tile as tile
from concourse.masks import make_identity
from concourse.bass_utils import run_bass_kernel_spmd

F32 = mybir.dt.float32
BF16 = mybir.dt.bfloat16
AF = mybir.ActivationFunctionType
OP = mybir.AluOpType

V = 472
DIM = 472
ED = 944
NS = 16
KC = 4
R = 30
DEPTH = 8
B = 2

NCORES = 8
EC = ED // NCORES            # 118
MT = DIM // 4                # 118 residual row-tile
VOUT = DIM // NCORES         # 59 lm_head rows per core

SETS = ['in'] + [f'l{i}' for i in range(DEPTH)] + ['out']


def _bf(x):
    return np.ascontiguousarray(np.asarray(x, np.float32).astype(ml_dtypes.bfloat16))


def _f32(x):
    return np.ascontiguousarray(np.asarray(x, np.float32))


def prep_core_inputs(core, inputs, L):
    e0 = core * EC
    e1 = e0 + EC
    T = B * L
    d = {}
    x = np.asarray(inputs['x'], np.float32)           # (B, L, 3, 3)
    d['x_rhs'] = _bf(x.reshape(T, 9).T)               # (9, T)
    pw = np.asarray(inputs['patch_w'], np.float32)[:, 0].reshape(V, 9)
    d['patch_lhsT'] = _bf(pw.T)                       # (9, DIM)
    d['patch_b'] = _f32(np.asarray(inputs['patch_b']).reshape(4, MT, 1))
    lm = np.asarray(inputs['lm_head_w'], np.float32)[core * VOUT:(core + 1) * VOUT]
    d['lm_lhsT'] = _bf(lm.T.reshape(4, MT, VOUT))     # (4, MT, VOUT)
    for s in SETS:
        if s == 'in':
            g = lambda n: np.asarray(inputs[f'in_{n}'], np.float32)
        elif s == 'out':
            g = lambda n: np.asarray(inputs[f'out_{n}'], np.float32)
        else:
            li = int(s[1:])
            g = lambda n, li=li: np.asarray(inputs[f'lay_{n}'], np.float32)[li]
        ip = g('inproj_w')
        d[f'{s}_wxi'] = _bf(ip[e0:e1].T.reshape(4, MT, EC))
        d[f'{s}_wz'] = _bf(ip[ED + e0:ED + e1].T.reshape(4, MT, EC))
        cw = g('conv_w')[e0:e1, 0]                    # (EC, KC)
        diag = np.zeros((KC, EC, EC), np.float32)
        idx = np.arange(EC)
        for k in range(KC):
            diag[k, idx, idx] = cw[:, k]
        d[f'{s}_conv'] = _bf(diag)
        d[f'{s}_convb'] = _f32(g('conv_b')[e0:e1].reshape(EC, 1))
        d[f'{s}_xp'] = _bf(g('xproj_w')[:, e0:e1].T)  # (EC, 62)
        d[f'{s}_dt'] = _bf(g('dt_w')[e0:e1].T)        # (R, EC)
        d[f'{s}_dtb'] = _f32(g('dt_b')[e0:e1].reshape(EC, 1))
        d[f'{s}_A'] = _f32(-np.exp(g('Alog')[e0:e1])) # (EC, NS)
        d[f'{s}_D'] = _f32(g('D')[e0:e1].reshape(EC, 1))
        d[f'{s}_op'] = _bf(g('outproj_w')[:, e0:e1].T)  # (EC, DIM)
    return d


class Ctx:
    pass


def build_kernel(L, repeat=1):
    T = B * L
    nt = min(512, L)
    ntiles = T // nt
    jts = L // nt                # per-sample tiles

    nc = bacc.Bacc(num_devices=NCORES)
    din = {}

    def dram_in(name, shape, dt):
        din[name] = nc.dram_tensor(name, list(shape), dt, kind="ExternalInput")

    dram_in('x_rhs', (9, T), BF16)
    dram_in('patch_lhsT', (9, DIM), BF16)
    dram_in('patch_b', (4, MT, 1), F32)
    dram_in('lm_lhsT', (4, MT, VOUT), BF16)
    for s in SETS:
        dram_in(f'{s}_wxi', (4, MT, EC), BF16)
        dram_in(f'{s}_wz', (4, MT, EC), BF16)
        dram_in(f'{s}_conv', (KC, EC, EC), BF16)
        dram_in(f'{s}_convb', (EC, 1), F32)
        dram_in(f'{s}_xp', (EC, R + 2 * NS), BF16)
        dram_in(f'{s}_dt', (R, EC), BF16)
        dram_in(f'{s}_dtb', (EC, 1), F32)
        dram_in(f'{s}_A', (EC, NS), F32)
        dram_in(f'{s}_D', (EC, 1), F32)
        dram_in(f'{s}_op', (EC, DIM), BF16)
    out_t = nc.dram_tensor('out', [VOUT, T], F32, kind="ExternalOutput")

    c = Ctx()
    c.nc, c.din, c.out_t = nc, din, out_t
    c.L, c.T, c.nt, c.ntiles, c.jts = L, T, nt, ntiles, jts

    with tile.TileContext(nc) as tc:
        c.tc = tc
        with (
            tc.tile_pool(name="wp", bufs=1) as wp,
            tc.tile_pool(name="hp", bufs=2) as hp,
            tc.tile_pool(name="ap", bufs=1) as ap_,
            tc.tile_pool(name="sp", bufs=2) as sp,
            tc.tile_pool(name="pp", bufs=4, space="PSUM") as pp,
            tc.tile_pool(name="yp", bufs=1, space="PSUM") as yp,
            tc.tile_pool(name="dp", bufs=1, space="DRAM") as dp,
        ):
            c.wp, c.hp, c.ap, c.sp, c.pp, c.yp, c.dp = wp, hp, ap_, sp, pp, yp, dp

            W = {}
            for name, t in din.items():
                shp = list(t.shape)
                if len(shp) == 3:
                    wt = wp.tile([shp[1], shp[0], shp[2]], t.dtype, tag=f"w_{name}")
                    nc.sync.dma_start(wt[:], t[:].rearrange("k m e -> m k e"))
                else:
                    wt = wp.tile(shp, t.dtype, tag=f"w_{name}")
                    nc.sync.dma_start(wt[:], t[:])
                W[name] = wt
            c.W = W
            ident = wp.tile([EC, EC], BF16, tag="ident")
            make_identity(nc, ident[:])
            c.ident = ident

            c.h_dram = dp.tile([DIM, T], F32, tag="h_dram")
            c.cc_in = dp.tile([124, T], BF16, tag="cc_in")
            c.cc_out = dp.tile([124, T], BF16, tag="cc_out")
            c.up_in = dp.tile([DIM, T], BF16, tag="up_in")
            c.up_out = dp.tile([DIM, T], BF16, tag="up_out")

            if repeat == 1:
                build_body(c)
            else:
                with tc.For_i(0, repeat, 1):
                    build_body(c)
    nc.compile()
    return nc


def load_h_rhs(c, j):
    """Stage residual tokens [j*nt:(j+1)*nt) as bf16 rhs k-tiles [MT, 4, nt]."""
    nc = c.nc
    hbj = c.hp.tile([MT, 4, c.nt], BF16, tag="hbj", bufs=3)
    nc.gpsimd.dma_start(
        hbj[:], c.h_dram[:, bass.ts(j, c.nt)].rearrange("(k m) t -> m k t", k=4))
    return hbj


def build_body(c):
    nc = c.nc
    W = c.W
    nt = c.nt

    # ---- patch embedding ----
    xr = c.ap.tile([9, c.T], BF16, tag="xr")
    nc.sync.dma_start(xr[:], c.din['x_rhs'][:])
    for m in range(4):
        for j in range(c.ntiles):
            ps = c.pp.tile([MT, nt], F32, tag="ps")
            nc.tensor.matmul(ps[:], W['patch_lhsT'][:, bass.ts(m, MT)],
                             xr[:, bass.ts(j, nt)], start=True, stop=True)
            st = c.hp.tile([MT, nt], F32, tag="hstage", bufs=4)
            nc.scalar.activation(st[:], ps[:], AF.Identity, bias=W['patch_b'][:, m, :])
            nc.sync.dma_start(c.h_dram[m * MT:(m + 1) * MT, bass.ts(j, nt)], st[:])

    # ---- blocks ----
    run_pair(c, ['in', 'in'], [False, True])
    for i in range(DEPTH):
        run_pair(c, [f'l{i}'], [False])
    run_pair(c, ['out', 'out'], [False, True])

    # ---- lm head ----
    for j in range(c.ntiles):
        hbj = load_h_rhs(c, j)
        ps = c.pp.tile([VOUT, nt], F32, tag="ps")
        for k in range(4):
            nc.tensor.matmul(ps[:], W['lm_lhsT'][:, k, :], hbj[:, k, :],
                             start=(k == 0), stop=(k == 3))
        ot = c.hp.tile([VOUT, nt], F32, tag="lmout")
        nc.vector.tensor_copy(ot[:], ps[:])
        nc.sync.dma_start(c.out_t[:, bass.ts(j, nt)], ot[:])


def run_pair(c, sets, revs):
    """One mid layer (sets=[s]) or a bidir pair (sets=[s,s], revs=[F,T])."""
    nc = c.nc
    W = c.W
    L, nt, jts = c.L, c.nt, c.jts
    s0 = sets[0]
    pair = len(sets) == 2

    # ---- in_proj (shared between directions; flip commutes with pointwise) ----
    xi = [c.ap.tile([EC, L + 6], BF16, tag=f"xi{b}", name=f"xi{b}") for b in range(B)]
    for b in range(B):
        nc.gpsimd.memset(xi[b][:, 0:3], 0.0)
        nc.gpsimd.memset(xi[b][:, 3 + L:], 0.0)
    sz = c.ap.tile([EC, c.T], BF16, tag="sz")
    for j in range(c.ntiles):
        b, jj = divmod(j, jts)
        hbj = load_h_rhs(c, j)
        ps = c.pp.tile([EC, nt], F32, tag="ps")
        for k in range(4):
            nc.tensor.matmul(ps[:], W[f'{s0}_wxi'][:, k, :], hbj[:, k, :],
                             start=(k == 0), stop=(k == 3))
        nc.scalar.activation(xi[b][:, 3 + jj * nt:3 + (jj + 1) * nt], ps[:], AF.Copy)
        ps2 = c.pp.tile([EC, nt], F32, tag="ps")
        for k in range(4):
            nc.tensor.matmul(ps2[:], W[f'{s0}_wz'][:, k, :], hbj[:, k, :],
                             start=(k == 0), stop=(k == 3))
        nc.scalar.activation(sz[:, bass.ts(j, nt)], ps2[:], AF.Silu)

    # ---- per-direction conv + xproj partial ----
    xcs = []
    for di, (s, rev) in enumerate(zip(sets, revs)):
        xc = c.ap.tile([EC, c.T], BF16, tag=f"xc{di}")
        for j in range(c.ntiles):
            b, jj = divmod(j, jts)
            ps = c.pp.tile([EC, nt], F32, tag="ps")
            for k in range(KC):
                off = (6 - k) if rev else k
                nc.tensor.matmul(ps[:], W[f'{s}_conv'][:, k, :],
                                 xi[b][:, jj * nt + off: jj * nt + off + nt],
                                 start=(k == 0), stop=(k == KC - 1))
            nc.scalar.activation(xc[:, bass.ts(j, nt)], ps[:], AF.Silu,
                                 bias=W[f'{s}_convb'][:])
        xcs.append(xc)
        for j in range(c.ntiles):
            ps = c.pp.tile([62, nt], F32, tag="ps")
            nc.tensor.matmul(ps[:], W[f'{s}_xp'][:], xc[:, bass.ts(j, nt)],
                             start=True, stop=True)
            st = c.hp.tile([62, nt], BF16, tag="dblst")
            nc.scalar.activation(st[:], ps[:], AF.Copy)
            nc.sync.dma_start(c.cc_in[62 * di:62 * (di + 1), bass.ts(j, nt)], st[:])

    # ---- merged AllReduce of x_proj partials ----
    rows = 124 if pair else 62
    nc.gpsimd.collective_compute(
        "AllReduce", OP.add, replica_groups=[list(range(NCORES))],
        ins=[c.cc_in[0:rows, :].opt()], outs=[c.cc_out[0:rows, :].opt()])

    # ---- per-direction: delta, scans, gating; accumulate y2sum ----
    y2sum = c.ap.tile([EC, c.T], BF16, tag="y2sum")
    for di, (s, rev) in enumerate(zip(sets, revs)):
        xc = xcs[di]
        dbl30 = c.ap.tile([R, c.T], BF16, tag="dbl30")
        nc.sync.dma_start(dbl30[:], c.cc_out[62 * di:62 * di + R, :])

        delta = c.ap.tile([EC, c.T], BF16, tag="delta")
        spt = c.ap.tile([EC, c.T], BF16, tag="spt")
        for j in range(c.ntiles):
            ps = c.pp.tile([EC, nt], F32, tag="ps")
            nc.tensor.matmul(ps[:], W[f'{s}_dt'][:], dbl30[:, bass.ts(j, nt)],
                             start=True, stop=True)
            # softplus(x) = ln(1 + e^x); no Softplus table on this arch.
            # Exp over all tiles first, then Ln, to minimize table reloads.
            nc.scalar.activation(spt[:, bass.ts(j, nt)], ps[:], AF.Exp,
                                 bias=W[f'{s}_dtb'][:])
        for j in range(c.ntiles):
            nc.scalar.activation(delta[:, bass.ts(j, nt)], spt[:, bass.ts(j, nt)],
                                 AF.Ln, bias=1.0)
        u = c.ap.tile([EC, c.T], BF16, tag="u")
        nc.vector.tensor_mul(u[:], delta[:], xc[:])

        for b in range(B):
            ypss = [c.yp.tile([EC, nt], F32, tag=f"y{jj}", name=f"yps{jj}") for jj in range(jts)]
            for n in range(NS):
                ag = c.sp.tile([EC, L], BF16, tag="ag")
                bg = c.sp.tile([EC, L], BF16, tag="bg")
                hg = c.sp.tile([EC, L], BF16, tag="hg")
                brep = c.sp.tile([EC, L], BF16, tag="brep")
                crep = c.sp.tile([EC, L], BF16, tag="crep")
                nc.scalar.activation(ag[:], delta[:, b * L:(b + 1) * L], AF.Exp,
                                     scale=W[f'{s}_A'][:, n:n + 1])
                rb = 62 * di + R + n
                nc.sync.dma_start(
                    brep[:, None, :],
                    c.cc_out[rb:rb + 1, b * L:(b + 1) * L].partition_broadcast(EC))
                nc.sync.dma_start(
                    crep[:, None, :],
                    c.cc_out[rb + NS:rb + NS + 1, b * L:(b + 1) * L].partition_broadcast(EC))
                nc.vector.tensor_mul(bg[:], u[:, b * L:(b + 1) * L], brep[:])
                if rev:
                    nc.vector.tensor_tensor_scan(
                        hg[:, ::-1], ag[:, ::-1], bg[:, ::-1], 0.0, OP.mult, OP.add)
                else:
                    nc.vector.tensor_tensor_scan(
                        hg[:], ag[:], bg[:], 0.0, OP.mult, OP.add)
                nc.gpsimd.tensor_mul(hg[:], hg[:], crep[:])
                for jj in range(jts):
                    nc.tensor.matmul(ypss[jj][:], c.ident[:], hg[:, bass.ts(jj, nt)],
                                     start=(n == 0), stop=(n == NS - 1))
            for jj in range(jts):
                j = b * jts + jj
                y2p = c.hp.tile([EC, nt], BF16, tag="y2p")
                nc.vector.scalar_tensor_tensor(
                    y2p[:], xc[:, bass.ts(j, nt)], W[f'{s}_D'][:], ypss[jj][:],
                    op0=OP.mult, op1=OP.add)
                if di == 0:
                    nc.vector.tensor_mul(y2sum[:, bass.ts(j, nt)], y2p[:],
                                         sz[:, bass.ts(j, nt)])
                else:
                    nc.vector.tensor_mul(y2p[:], y2p[:], sz[:, bass.ts(j, nt)])
                    nc.vector.tensor_add(y2sum[:, bass.ts(j, nt)],
                                         y2sum[:, bass.ts(j, nt)], y2p[:])

    # ---- out_proj partial on y2sum ----
    for m in range(4):
        for j in range(c.ntiles):
            ps = c.pp.tile([MT, nt], F32, tag="ps")
            nc.tensor.matmul(ps[:], W[f'{s0}_op'][:, bass.ts(m, MT)],
                             y2sum[:, bass.ts(j, nt)], start=True, stop=True)
            st = c.hp.tile([MT, nt], BF16, tag="opst")
            nc.scalar.activation(st[:], ps[:], AF.Copy)
            nc.sync.dma_start(c.up_in[m * MT:(m + 1) * MT, bass.ts(j, nt)], st[:])

    nc.gpsimd.collective_compute(
        "AllReduce", OP.add, replica_groups=[list(range(NCORES))],
        ins=[c.up_in[:].opt()], outs=[c.up_out[:].opt()])

    # ---- residual update: h_dram += up_out ----
    for m in range(4):
        for j in range(c.ntiles):
            hs = c.hp.tile([MT, nt], F32, tag="hstage", bufs=4)
            nc.sync.dma_start(hs[:], c.h_dram[m * MT:(m + 1) * MT, bass.ts(j, nt)])
            us = c.hp.tile([MT, nt], BF16, tag="ustage", bufs=4)
            nc.sync.dma_start(us[:], c.up_out[m * MT:(m + 1) * MT, bass.ts(j, nt)])
            nc.vector.tensor_add(hs[:], hs[:], us[:])
            nc.sync.dma_start(c.h_dram[m * MT:(m + 1) * MT, bass.ts(j, nt)], hs[:])


_KERNEL_CACHE = {}


def get_kernel(L, repeat=1):
    key = (L, repeat)
    if key not in _KERNEL_CACHE:
        _KERNEL_CACHE[key] = build_kernel(L, repeat)
    return _KERNEL_CACHE[key]


def kernel(**inputs):
    L = int(np.asarray(inputs['x']).shape[1])
    nc = get_kernel(L)
    in_maps = [prep_core_inputs(cc, inputs, L) for cc in range(NCORES)]
    res = run_bass_kernel_spmd(nc, in_maps, list(range(NCORES)))
    outs = [np.asarray(res.results[cc]['out'], np.float32) for cc in range(NCORES)]
    full = np.concatenate(outs, axis=0)                       # (V, T)
    return np.ascontiguousarray(full.reshape(V, B, L).transpose(1, 2, 0))



# revision 17
# speedup vs baseline: 1.7141x; 1.7141x over previous
"""Bass/Trainium2 kernel for nn_BysMamba (bidirectional + stacked Mamba LM).

Sharding: DP2 x TP4. Cores 0-3 own sample 0, cores 4-7 sample 1 (full
L=2048 sequence each). Within a sample group, d_inner ED=944 is split 4
ways (236 channels/core) for scan/gating/out_proj, while the x-branch of
in_proj, the causal conv and x_proj are computed redundantly on the full
944 channels so dbl/B/C/delta need no collective. The full-channel layout
is PERMUTED per core (own shard first) so the SPMD program is uniform.
Per layer the only collectives are a 4-way ReduceScatter of out_proj
partials and a 4-way AllGather of the updated residual (bf16), each split
in two token halves so they overlap the other half's scan compute. The
residual h lives in SBUF: fp32 master of this core's 118-row DIM shard +
full bf16 copy from the AllGather. exp(A_n*delta) exploits the S4D-real
structure (A_n ~ -(n+1)): low n direct on Act, high n chained multiply by
q = exp(-delta) on DVE. Scans split across DVE and Pool; y contracts over
the 16 states via identity matmuls accumulating in PSUM.
"""
import sys
sys.path.insert(0, '/opt/trn_rl_repo')

import numpy as np
import ml_dtypes

import concourse.bass as bass
from concourse import bacc
import concourse.mybir as mybir
import concourse.tile as tile
from concourse.masks import make_identity
from concourse.bass_utils import run_bass_kernel_spmd

F32 = mybir.dt.float32
BF16 = mybir.dt.bfloat16
AF = mybir.ActivationFunctionType
OP = mybir.AluOpType

V = 472
DIM = 472
ED = 944
NS = 16
KC = 4
R = 30
DEPTH = 8
B = 2

NCORES = 8
TPG = 4                      # tensor-parallel group size
P = 118                      # partition tile
KT = DIM // P                # 4 k-tiles over DIM
MT_FULL = ED // P            # 8 channel tiles (full)
EC = ED // TPG               # 236 channels per core
MT_SH = EC // P              # 2 channel tiles (shard)
NT = 512                     # psum column tile

ACT_N = 10                   # n < ACT_N: ag via Act exp; else DVE chain

SETS = ['in'] + [f'l{i}' for i in range(DEPTH)] + ['out']


def _bf(x):
    return np.ascontiguousarray(np.asarray(x, np.float32).astype(ml_dtypes.bfloat16))


def _f32(x):
    return np.ascontiguousarray(np.asarray(x, np.float32))


def prep_core_inputs(core, inputs, L):
    s, r = divmod(core, TPG)
    e0 = r * EC
    perm = np.r_[e0:e0 + EC, 0:e0, e0 + EC:ED]         # own shard first
    d = {}
    x = np.asarray(inputs['x'], np.float32)[s]         # (L, 3, 3)
    d['x_rhs'] = _bf(x.reshape(L, 9).T)                # (9, L)
    pw = np.asarray(inputs['patch_w'], np.float32)[:, 0].reshape(V, 9)
    d['patch_lhsT'] = _bf(pw.T.reshape(9, KT, P))      # (9, 4, 118)
    d['patch_b'] = _f32(np.asarray(inputs['patch_b']).reshape(KT, P, 1).transpose(1, 0, 2))
    d['patch_lhsT_sh'] = _bf(pw.T[:, r * P:(r + 1) * P])          # (9, 118)
    d['patch_b_sh'] = _f32(np.asarray(inputs['patch_b'])[r * P:(r + 1) * P].reshape(P, 1))
    lm = np.asarray(inputs['lm_head_w'], np.float32)[r * P:(r + 1) * P]   # (118, 472)
    d['lm_lhsT'] = _bf(lm.T.reshape(KT, P, P).transpose(1, 0, 2))         # (118,4,118)
    for snm in SETS:
        if snm == 'in':
            g = lambda n: np.asarray(inputs[f'in_{n}'], np.float32)
        elif snm == 'out':
            g = lambda n: np.asarray(inputs[f'out_{n}'], np.float32)
        else:
            li = int(snm[1:])
            g = lambda n, li=li: np.asarray(inputs[f'lay_{n}'], np.float32)[li]
        ip = g('inproj_w')
        # xi part: FULL 944 rows, permuted; lhsT (472,944)->(118,4,8,118)
        wxi = ip[:ED][perm].T.reshape(KT, P, MT_FULL, P).transpose(1, 0, 2, 3)
        d[f'{snm}_wxi'] = _bf(wxi)
        wz = ip[ED + e0:ED + e0 + EC].T.reshape(KT, P, MT_SH, P).transpose(1, 0, 2, 3)
        d[f'{snm}_wz'] = _bf(wz)
        cw = g('conv_w')[:, 0][perm]                    # (944, 4) permuted
        diag = np.zeros((KC, MT_FULL, P, P), np.float32)
        idx = np.arange(P)
        for k in range(KC):
            for mt in range(MT_FULL):
                diag[k, mt, idx, idx] = cw[mt * P:(mt + 1) * P, k]
        d[f'{snm}_conv'] = _bf(diag.transpose(2, 0, 1, 3))   # (118,4,8,118)
        d[f'{snm}_convb'] = _f32(g('conv_b')[perm].reshape(MT_FULL, P, 1).transpose(1, 0, 2))
        xpw = g('xproj_w')[:, perm]                     # (62, 944)
        rowp = np.r_[0:R, [R + i // 2 + NS * (i % 2) for i in range(2 * NS)]]
        xpw = xpw[rowp]                                 # B/C interleaved pairs
        d[f'{snm}_xp'] = _bf(xpw.T.reshape(MT_FULL, P, R + 2 * NS).transpose(1, 0, 2))
        d[f'{snm}_dt'] = _bf(g('dt_w')[e0:e0 + EC].T.reshape(R, MT_SH, P))  # (30,2,118)
        d[f'{snm}_dtb'] = _f32(-g('dt_b')[e0:e0 + EC].reshape(MT_SH, P, 1).transpose(1, 0, 2))
        d[f'{snm}_A'] = _f32(np.exp(g('Alog')[e0:e0 + EC]).reshape(MT_SH, P, NS).transpose(1, 0, 2))
        d[f'{snm}_D'] = _f32(g('D')[e0:e0 + EC].reshape(MT_SH, P, 1).transpose(1, 0, 2))
        op = g('outproj_w')[:, e0:e0 + EC].T            # (236, 472)
        d[f'{snm}_op'] = _bf(op.reshape(MT_SH, P, KT, P).transpose(1, 0, 2, 3))
    return d


class Ctx:
    pass


def build_kernel(L, repeat=1):
    HL = L // 2                  # half length
    jh = HL // NT                # 512-tiles per half

    nc = bacc.Bacc(num_devices=NCORES)
    din = {}

    def dram_in(name, shape, dt):
        din[name] = nc.dram_tensor(name, list(shape), dt, kind="ExternalInput")

    dram_in('x_rhs', (9, L), BF16)
    dram_in('patch_lhsT', (9, KT, P), BF16)
    dram_in('patch_b', (P, KT, 1), F32)
    dram_in('patch_lhsT_sh', (9, P), BF16)
    dram_in('patch_b_sh', (P, 1), F32)
    dram_in('lm_lhsT', (P, KT, P), BF16)
    for s in SETS:
        dram_in(f'{s}_wxi', (P, KT, MT_FULL, P), BF16)
        dram_in(f'{s}_wz', (P, KT, MT_SH, P), BF16)
        dram_in(f'{s}_conv', (P, KC, MT_FULL, P), BF16)
        dram_in(f'{s}_convb', (P, MT_FULL, 1), F32)
        dram_in(f'{s}_xp', (P, MT_FULL, R + 2 * NS), BF16)
        dram_in(f'{s}_dt', (R, MT_SH, P), BF16)
        dram_in(f'{s}_dtb', (P, MT_SH, 1), F32)
        dram_in(f'{s}_A', (P, MT_SH, NS), F32)
        dram_in(f'{s}_D', (P, MT_SH, 1), F32)
        dram_in(f'{s}_op', (P, MT_SH, KT, P), BF16)
    out_t = nc.dram_tensor('out', [P, L], F32, kind="ExternalOutput")

    c = Ctx()
    c.nc, c.din, c.out_t = nc, din, out_t
    c.L, c.HL, c.jh = L, HL, jh
    c.groups = [[0, 1, 2, 3], [4, 5, 6, 7]]

    with tile.TileContext(nc) as tc:
        c.tc = tc
        with (
            tc.tile_pool(name="wp", bufs=1) as wp,      # streamed per-set weights
            tc.tile_pool(name="gp", bufs=1) as gp,      # persistent globals + activations
            tc.tile_pool(name="sp", bufs=3) as sp,      # scan transients
            tc.tile_pool(name="hp", bufs=4) as hp,      # staging
            tc.tile_pool(name="pp", bufs=4, space="PSUM") as pp,
            tc.tile_pool(name="yp", bufs=1, space="PSUM") as yp,
            tc.tile_pool(name="dp", bufs=1, space="DRAM") as dp,
        ):
            c.wp, c.gp, c.sp, c.hp, c.pp, c.yp, c.dp = wp, gp, sp, hp, pp, yp, dp

            # globals
            G = {}
            for nm in ('x_rhs', 'patch_lhsT', 'patch_b', 'patch_lhsT_sh',
                       'patch_b_sh', 'lm_lhsT'):
                t = din[nm]
                gt = gp.tile(list(t.shape), t.dtype, tag=f"g_{nm}")
                nc.sync.dma_start(gt[:], t[:])
                G[nm] = gt
            c.G = G
            ident = gp.tile([P, P], BF16, tag="ident")
            make_identity(nc, ident[:])
            c.ident = ident

            # persistent activations
            c.hbf = gp.tile([P, KT, L], BF16, tag="hbf")        # full h, bf16
            c.hms = gp.tile([P, L], F32, tag="hms")             # own DIM-shard master
            c.xi = gp.tile([P, MT_FULL, L + 6], BF16, tag="xi")
            c.sz = gp.tile([P, MT_SH, L], BF16, tag="sz")
            c.xc = gp.tile([P, MT_FULL, L], BF16, tag="xc")
            c.dblS = gp.tile([R + 2 * NS, L], BF16, tag="dblS")
            c.delta = gp.tile([P, MT_SH, L], BF16, tag="delta")
            c.u = gp.tile([P, MT_SH, L], BF16, tag="u")
            c.y2sum = gp.tile([P, MT_SH, L], BF16, tag="y2sum")
            c.state = gp.tile([P, MT_SH, NS], F32, tag="state")

            # zero the conv pads once
            for mt in range(MT_FULL):
                nc.gpsimd.memset(c.xi[:, mt, 0:3], 0.0)
                nc.gpsimd.memset(c.xi[:, mt, 3 + L:], 0.0)

            # DRAM staging for collectives (per half)
            c.bc_dram = [dp.tile([2 * NS, HL], BF16, tag=f"bc_dram{h}", name=f"bc_dram{h}")
                         for h in range(2)]
            c.up_in = [dp.tile([DIM, HL], BF16, tag=f"up_in{h}", name=f"up_in{h}")
                       for h in range(2)]
            c.up_rs = [dp.tile([P, HL], BF16, tag=f"up_rs{h}", name=f"up_rs{h}")
                       for h in range(2)]
            c.hag_in = [dp.tile([P, HL], BF16, tag=f"hag_in{h}", name=f"hag_in{h}")
                        for h in range(2)]
            c.hag_out = [dp.tile([DIM, HL], BF16, tag=f"hag_out{h}", name=f"hag_out{h}")
                         for h in range(2)]

            if repeat == 1:
                build_body(c)
            else:
                with tc.For_i(0, repeat, 1):
                    build_body(c)
    nc.compile()
    return nc


def load_set_weights(c, s):
    nc = c.nc
    W = {}
    for suff in ('wxi', 'wz', 'conv', 'convb', 'xp', 'dt', 'dtb', 'A', 'D', 'op'):
        t = c.din[f'{s}_{suff}']
        wt = c.wp.tile(list(t.shape), t.dtype, tag=f"w_{suff}")
        nc.sync.dma_start(wt[:], t[:])
        W[suff] = wt
    return W


def build_body(c):
    nc = c.nc
    L = c.L
    G = c.G

    # ---- patch embedding: full h bf16 + own fp32 shard ----
    for m in range(KT):
        for j in range(L // NT):
            ps = c.pp.tile([P, NT], F32, tag="ps")
            nc.tensor.matmul(ps[:], G['patch_lhsT'][:, m, :],
                             G['x_rhs'][:, bass.ts(j, NT)], start=True, stop=True)
            nc.scalar.activation(c.hbf[:, m, bass.ts(j, NT)], ps[:], AF.Identity,
                                 bias=G['patch_b'][:, m, :])
    for j in range(L // NT):
        ps = c.pp.tile([P, NT], F32, tag="ps")
        nc.tensor.matmul(ps[:], G['patch_lhsT_sh'][:],
                         G['x_rhs'][:, bass.ts(j, NT)], start=True, stop=True)
        nc.scalar.activation(c.hms[:, bass.ts(j, NT)], ps[:], AF.Identity,
                             bias=G['patch_b_sh'][:])

    # ---- blocks ----
    run_block(c, 'in', pair=True)
    for i in range(DEPTH):
        run_block(c, f'l{i}', pair=False)
    run_block(c, 'out', pair=True)

    # ---- lm head ----
    for j in range(L // NT):
        ps = c.pp.tile([P, NT], F32, tag="ps")
        for k in range(KT):
            nc.tensor.matmul(ps[:], G['lm_lhsT'][:, k, :],
                             c.hbf[:, k, bass.ts(j, NT)],
                             start=(k == 0), stop=(k == KT - 1))
        ot = c.hp.tile([P, NT], F32, tag="lmout", bufs=2)
        nc.vector.tensor_copy(ot[:], ps[:])
        nc.sync.dma_start(c.out_t[:, bass.ts(j, NT)], ot[:])


def prep_half(c, W, h, rev):
    """conv + xproj + delta + u for token half h (in_proj already done)."""
    nc = c.nc
    HL, jh = c.HL, c.jh
    t0 = h * HL

    # conv (full channels) -> silu -> xc
    for mt in range(MT_FULL):
        for j in range(jh):
            ps = c.pp.tile([P, NT], F32, tag="ps")
            for k in range(KC):
                off = (6 - k) if rev else k
                nc.tensor.matmul(ps[:], W['conv'][:, k, mt, :],
                                 c.xi[:, mt, t0 + j * NT + off: t0 + j * NT + off + NT],
                                 start=(k == 0), stop=(k == KC - 1))
            nc.scalar.activation(c.xc[:, mt, t0 + j * NT: t0 + (j + 1) * NT], ps[:],
                                 AF.Silu, bias=W['convb'][:, mt, :])

    # xproj (full, local): dbl[62, HL]
    for j in range(jh):
        psf = c.pp.tile([P, NT], F32, tag="ps")
        ps = psf[0:R + 2 * NS, :]
        for kt in range(MT_FULL):
            nc.tensor.matmul(ps, W['xp'][:, kt, :],
                             c.xc[:, kt, t0 + j * NT: t0 + (j + 1) * NT],
                             start=(kt == 0), stop=(kt == MT_FULL - 1))
        nc.vector.tensor_copy(c.dblS[:, t0 + j * NT: t0 + (j + 1) * NT], ps)

    nc.sync.dma_start(c.bc_dram[h][:], c.dblS[R:R + 2 * NS, t0:t0 + HL])

    # q = sigmoid(-(dtx+dtb)); delta tile holds lnq = -softplus(dtx+dtb)
    for mt in range(MT_SH):
        for j in range(jh):
            ps = c.pp.tile([P, NT], F32, tag="ps")
            nc.tensor.matmul(ps[:], W['dt'][:, mt, :],
                             c.dblS[0:R, t0 + j * NT: t0 + (j + 1) * NT],
                             start=True, stop=True)
            nc.scalar.activation(c.u[:, mt, t0 + j * NT: t0 + (j + 1) * NT],
                                 ps[:], AF.Sigmoid, bias=W['dtb'][:, mt, :],
                                 scale=-1.0)
    for mt in range(MT_SH):
        nc.scalar.activation(c.delta[:, mt, t0:t0 + HL], c.u[:, mt, t0:t0 + HL],
                             AF.Ln)
    # u = (-lnq) * xc_shard = softplus * xc
    for mt in range(MT_SH):
        nc.vector.scalar_tensor_tensor(c.u[:, mt, t0:t0 + HL],
                                       c.delta[:, mt, t0:t0 + HL], -1.0,
                                       c.xc[:, mt, t0:t0 + HL],
                                       op0=OP.mult, op1=OP.mult)


def scan_half(c, W, h, rev, first_half, di):
    """Selective scan + gating for token half h of direction di."""
    nc = c.nc
    HL, jh = c.HL, c.jh
    t0 = h * HL

    yps = [c.yp.tile([P, HL], F32, tag=f"yacc{mt}", name=f"yacc{mt}")
           for mt in range(MT_SH)]
    for n in range(NS):
        bcrep = c.sp.tile([P, 2, HL], BF16, tag="bcrep", bufs=2)
        nc.sync.dma_start(bcrep[:, None, :, :],
                          c.bc_dram[h][2 * n:2 * n + 2, :].partition_broadcast(P))
        brep = bcrep[:, 0, :]
        crep = bcrep[:, 1, :]
        bgeng = nc.gpsimd if (n % 3 == 1) else nc.vector
        for mt in range(MT_SH):
            ag = c.sp.tile([P, HL], BF16, tag=f"ag{mt}", bufs=2)
            nc.scalar.activation(ag[:], c.delta[:, mt, t0:t0 + HL], AF.Exp,
                                 scale=W['A'][:, mt, n:n + 1])

            bg = c.sp.tile([P, HL], BF16, tag=f"bg{mt}", bufs=2)
            bgeng.tensor_mul(bg[:], c.u[:, mt, t0:t0 + HL], brep)

            hg = c.sp.tile([P, HL], BF16, tag=f"hg{mt}", bufs=2)
            init = 0.0 if first_half else c.state[:, mt, n:n + 1]
            eng = nc.vector
            if rev:
                eng.tensor_tensor_scan(hg[:, ::-1], ag[:, ::-1], bg[:, ::-1],
                                       init, OP.mult, OP.add)
                if first_half:
                    nc.vector.tensor_copy(c.state[:, mt, n:n + 1], hg[:, 0:1])
            else:
                eng.tensor_tensor_scan(hg[:], ag[:], bg[:], init, OP.mult, OP.add)
                if first_half:
                    nc.vector.tensor_copy(c.state[:, mt, n:n + 1], hg[:, HL - 1:HL])

            hgc = c.sp.tile([P, HL], BF16, tag=f"bg{mt}", bufs=2, name=f"hgc{mt}")
            bgeng.tensor_mul(hgc[:], hg[:], crep)
            for ch in range(jh):
                nc.tensor.matmul(yps[mt][:, bass.ts(ch, NT)], c.ident[:],
                                 hgc[:, bass.ts(ch, NT)],
                                 start=(n == 0), stop=(n == NS - 1))

    # gating: y2 = yacc + D*xc ; y2s = y2 * sz (accumulate over directions)
    for mt in range(MT_SH):
        y2 = c.hp.tile([P, HL], BF16, tag="y2", bufs=2)
        nc.vector.scalar_tensor_tensor(y2[:], c.xc[:, mt, t0:t0 + HL],
                                       W['D'][:, mt, :], yps[mt][:],
                                       op0=OP.mult, op1=OP.add)
        if di == 0:
            nc.vector.tensor_mul(c.y2sum[:, mt, t0:t0 + HL], y2[:],
                                 c.sz[:, mt, t0:t0 + HL])
        else:
            y3 = c.hp.tile([P, HL], BF16, tag="y3", bufs=2)
            nc.gpsimd.tensor_mul(y3[:], y2[:], c.sz[:, mt, t0:t0 + HL])
            nc.vector.tensor_add(c.y2sum[:, mt, t0:t0 + HL],
                                 c.y2sum[:, mt, t0:t0 + HL], y3[:])


def residual_update(c, W, h):
    """out_proj partials for half h -> RS -> h master update -> AG -> hbf."""
    nc = c.nc
    HL, jh = c.HL, c.jh
    t0 = h * HL
    for m in range(KT):
        st = c.hp.tile([P, HL], BF16, tag="opst", bufs=2)
        for j in range(jh):
            ps = c.pp.tile([P, NT], F32, tag="ps")
            for kt in range(MT_SH):
                nc.tensor.matmul(ps[:], W['op'][:, kt, m, :],
                                 c.y2sum[:, kt, t0 + j * NT: t0 + (j + 1) * NT],
                                 start=(kt == 0), stop=(kt == MT_SH - 1))
            nc.scalar.activation(st[:, bass.ts(j, NT)], ps[:], AF.Copy)
        nc.sync.dma_start(c.up_in[h][m * P:(m + 1) * P, :], st[:])

    nc.gpsimd.collective_compute(
        "ReduceScatter", OP.add, replica_groups=c.groups,
        ins=[c.up_in[h][:].opt()], outs=[c.up_rs[h][:].opt()])

    ust = c.hp.tile([P, HL], BF16, tag="ust", bufs=1)
    nc.sync.dma_start(ust[:], c.up_rs[h][:])
    hst = c.hp.tile([P, HL], BF16, tag="hst", bufs=1)
    nc.vector.tensor_add(hst[:], c.hms[:, t0:t0 + HL], ust[:])
    nc.sync.dma_start(c.hag_in[h][:], hst[:])
    nc.gpsimd.tensor_add(c.hms[:, t0:t0 + HL], c.hms[:, t0:t0 + HL], ust[:])

    nc.gpsimd.collective_compute(
        "AllGather", OP.bypass, replica_groups=c.groups,
        ins=[c.hag_in[h][:].opt()], outs=[c.hag_out[h][:].opt()])
    nc.gpsimd.dma_start(c.hbf[:, :, t0:t0 + HL],
                        c.hag_out[h][:].rearrange("(k m) t -> m k t", k=KT))


def dirs_of(pair):
    return [(False, 0), (True, 1)] if pair else [(False, 0)]


def in_proj_half(c, W, h):
    nc = c.nc
    HL, jh = c.HL, c.jh
    for j in range(h * jh, (h + 1) * jh):
        for mt in range(MT_FULL):
            ps = c.pp.tile([P, NT], F32, tag="ps")
            for k in range(KT):
                nc.tensor.matmul(ps[:], W['wxi'][:, k, mt, :],
                                 c.hbf[:, k, bass.ts(j, NT)],
                                 start=(k == 0), stop=(k == KT - 1))
            dst = c.xi[:, mt, 3 + j * NT: 3 + (j + 1) * NT]
            if mt % 2 == 0:
                nc.vector.tensor_copy(dst, ps[:])
            else:
                nc.scalar.activation(dst, ps[:], AF.Copy)
        for mt in range(MT_SH):
            ps = c.pp.tile([P, NT], F32, tag="ps")
            for k in range(KT):
                nc.tensor.matmul(ps[:], W['wz'][:, k, mt, :],
                                 c.hbf[:, k, bass.ts(j, NT)],
                                 start=(k == 0), stop=(k == KT - 1))
            nc.scalar.activation(c.sz[:, mt, bass.ts(j, NT)], ps[:], AF.Silu)


def run_block(c, s, pair):
    nc = c.nc
    L = c.L
    W = load_set_weights(c, s)

    for rev, di in dirs_of(pair):
        if not rev:
            # forward direction: interleave in_proj per half with scans
            for h in (0, 1):
                in_proj_half(c, W, h)
                prep_half(c, W, h, rev)
                scan_half(c, W, h, rev, h == 0, di)
                if not pair:
                    residual_update(c, W, h)
        else:
            # reverse direction of a pair: xi already complete; h1 first
            prep_half(c, W, 1, rev)
            scan_half(c, W, 1, rev, True, di)
            residual_update(c, W, 1)
            prep_half(c, W, 0, rev)
            scan_half(c, W, 0, rev, False, di)
            residual_update(c, W, 0)


_KERNEL_CACHE = {}


def get_kernel(L, repeat=1):
    key = (L, repeat)
    if key not in _KERNEL_CACHE:
        _KERNEL_CACHE[key] = build_kernel(L, repeat)
    return _KERNEL_CACHE[key]


def kernel(**inputs):
    L = int(np.asarray(inputs['x']).shape[1])
    nc = get_kernel(L)
    in_maps = [prep_core_inputs(cc, inputs, L) for cc in range(NCORES)]
    res = run_bass_kernel_spmd(nc, in_maps, list(range(NCORES)))
    outs = [np.asarray(res.results[cc]['out'], np.float32) for cc in range(NCORES)]
    full = []
    for srow in range(B):
        sm = np.concatenate(outs[srow * TPG:(srow + 1) * TPG], axis=0)  # (472, L)
        full.append(sm.T)
    return np.ascontiguousarray(np.stack(full, axis=0))


# revision 38
# speedup vs baseline: 1.8037x; 1.0522x over previous
"""Bass/Trainium2 kernel for nn_BysMamba (bidirectional + stacked Mamba LM).

Sharding: DP2 x TP4. Cores 0-3 own sample 0, cores 4-7 sample 1 (full
L=2048 sequence each). Within a sample group, d_inner ED=944 is split 4
ways (236 channels/core) for scan/gating/out_proj, while the x-branch of
in_proj, the causal conv and x_proj are computed redundantly on the full
944 channels so dbl/B/C/delta need no collective. The full-channel layout
is PERMUTED per core (own shard first) so the SPMD program is uniform.
Per layer the only collectives are a 4-way ReduceScatter of out_proj
partials and a 4-way AllGather of the updated residual (bf16), each split
in two token halves so they overlap the other half's scan compute. The
residual h lives in SBUF: fp32 master of this core's 118-row DIM shard +
full bf16 copy from the AllGather. exp(A_n*delta) exploits the S4D-real
structure (A_n ~ -(n+1)): low n direct on Act, high n chained multiply by
q = exp(-delta) on DVE. Scans split across DVE and Pool; y contracts over
the 16 states via identity matmuls accumulating in PSUM.
"""
import sys
sys.path.insert(0, '/opt/trn_rl_repo')

import numpy as np
import ml_dtypes

import concourse.bass as bass
from concourse import bacc
import concourse.mybir as mybir
import concourse.tile as tile
from concourse.masks import make_identity
from concourse.bass_utils import run_bass_kernel_spmd

F32 = mybir.dt.float32
BF16 = mybir.dt.bfloat16
AF = mybir.ActivationFunctionType
OP = mybir.AluOpType

V = 472
DIM = 472
ED = 944
NS = 16
KC = 4
R = 30
DEPTH = 8
B = 2

NCORES = 8
TPG = 4                      # tensor-parallel group size
P = 118                      # partition tile
KT = DIM // P                # 4 k-tiles over DIM
MT_FULL = ED // P            # 8 channel tiles (full)
EC = ED // TPG               # 236 channels per core
MT_SH = EC // P              # 2 channel tiles (shard)
NT = 512                     # psum column tile

ACT_N = 10                   # n < ACT_N: ag via Act exp; else DVE chain

SETS = ['in'] + [f'l{i}' for i in range(DEPTH)] + ['out']


def _bf(x):
    return np.ascontiguousarray(np.asarray(x, np.float32).astype(ml_dtypes.bfloat16))


def _f32(x):
    return np.ascontiguousarray(np.asarray(x, np.float32))


def prep_core_inputs(core, inputs, L):
    s, r = divmod(core, TPG)
    e0 = r * EC
    perm = np.r_[e0:e0 + EC, 0:e0, e0 + EC:ED]         # own shard first
    d = {}
    x = np.asarray(inputs['x'], np.float32)[s]         # (L, 3, 3)
    d['x_rhs'] = _bf(x.reshape(L, 9).T)                # (9, L)
    pw = np.asarray(inputs['patch_w'], np.float32)[:, 0].reshape(V, 9)
    d['patch_lhsT'] = _bf(pw.T.reshape(9, KT, P))      # (9, 4, 118)
    d['patch_b'] = _f32(np.asarray(inputs['patch_b']).reshape(KT, P, 1).transpose(1, 0, 2))
    d['patch_lhsT_sh'] = _bf(pw.T[:, r * P:(r + 1) * P])          # (9, 118)
    d['patch_b_sh'] = _f32(np.asarray(inputs['patch_b'])[r * P:(r + 1) * P].reshape(P, 1))
    lm = np.asarray(inputs['lm_head_w'], np.float32)[r * P:(r + 1) * P]   # (118, 472)
    d['lm_lhsT'] = _bf(lm.T.reshape(KT, P, P).transpose(1, 0, 2))         # (118,4,118)
    for snm in SETS:
        if snm == 'in':
            g = lambda n: np.asarray(inputs[f'in_{n}'], np.float32)
        elif snm == 'out':
            g = lambda n: np.asarray(inputs[f'out_{n}'], np.float32)
        else:
            li = int(snm[1:])
            g = lambda n, li=li: np.asarray(inputs[f'lay_{n}'], np.float32)[li]
        ip = g('inproj_w')
        # xi part: FULL 944 rows, permuted; lhsT (472,944)->(118,4,8,118)
        wxi = ip[:ED][perm].T.reshape(KT, P, MT_FULL, P).transpose(1, 0, 2, 3)
        d[f'{snm}_wxi'] = _bf(wxi)
        wz = ip[ED + e0:ED + e0 + EC].T.reshape(KT, P, MT_SH, P).transpose(1, 0, 2, 3)
        d[f'{snm}_wz'] = _bf(wz)
        cw = g('conv_w')[:, 0][perm]                    # (944, 4) permuted
        diag = np.zeros((KC, MT_FULL, P, P), np.float32)
        idx = np.arange(P)
        for k in range(KC):
            for mt in range(MT_FULL):
                diag[k, mt, idx, idx] = cw[mt * P:(mt + 1) * P, k]
        d[f'{snm}_conv'] = _bf(diag.transpose(2, 0, 1, 3))   # (118,4,8,118)
        d[f'{snm}_convb'] = _f32(g('conv_b')[perm].reshape(MT_FULL, P, 1).transpose(1, 0, 2))
        xpw = g('xproj_w')[:, perm]                     # (62, 944)
        rowp = np.r_[0:R, [R + i // 2 + NS * (i % 2) for i in range(2 * NS)]]
        xpw = xpw[rowp]                                 # B/C interleaved pairs
        d[f'{snm}_xp'] = _bf(xpw.T.reshape(MT_FULL, P, R + 2 * NS).transpose(1, 0, 2))
        d[f'{snm}_dt'] = _bf(g('dt_w')[e0:e0 + EC].T.reshape(R, MT_SH, P))  # (30,2,118)
        d[f'{snm}_dtb'] = _f32(-g('dt_b')[e0:e0 + EC].reshape(MT_SH, P, 1).transpose(1, 0, 2))
        d[f'{snm}_A'] = _f32(np.exp(g('Alog')[e0:e0 + EC]).reshape(MT_SH, P, NS).transpose(1, 0, 2))
        d[f'{snm}_D'] = _f32(g('D')[e0:e0 + EC].reshape(MT_SH, P, 1).transpose(1, 0, 2))
        op = g('outproj_w')[:, e0:e0 + EC].T            # (236, 472)
        d[f'{snm}_op'] = _bf(op.reshape(MT_SH, P, KT, P).transpose(1, 0, 2, 3))
    return d


class Ctx:
    pass


def build_kernel(L, repeat=1):
    HL = L // 2                  # half length
    jh = HL // NT                # 512-tiles per half

    nc = bacc.Bacc(num_devices=NCORES)
    din = {}

    def dram_in(name, shape, dt):
        din[name] = nc.dram_tensor(name, list(shape), dt, kind="ExternalInput")

    dram_in('x_rhs', (9, L), BF16)
    dram_in('patch_lhsT', (9, KT, P), BF16)
    dram_in('patch_b', (P, KT, 1), F32)
    dram_in('patch_lhsT_sh', (9, P), BF16)
    dram_in('patch_b_sh', (P, 1), F32)
    dram_in('lm_lhsT', (P, KT, P), BF16)
    for s in SETS:
        dram_in(f'{s}_wxi', (P, KT, MT_FULL, P), BF16)
        dram_in(f'{s}_wz', (P, KT, MT_SH, P), BF16)
        dram_in(f'{s}_conv', (P, KC, MT_FULL, P), BF16)
        dram_in(f'{s}_convb', (P, MT_FULL, 1), F32)
        dram_in(f'{s}_xp', (P, MT_FULL, R + 2 * NS), BF16)
        dram_in(f'{s}_dt', (R, MT_SH, P), BF16)
        dram_in(f'{s}_dtb', (P, MT_SH, 1), F32)
        dram_in(f'{s}_A', (P, MT_SH, NS), F32)
        dram_in(f'{s}_D', (P, MT_SH, 1), F32)
        dram_in(f'{s}_op', (P, MT_SH, KT, P), BF16)
    out_t = nc.dram_tensor('out', [P, L], F32, kind="ExternalOutput")

    c = Ctx()
    c.nc, c.din, c.out_t = nc, din, out_t
    c.L, c.HL, c.jh = L, HL, jh
    c.groups = [[0, 1, 2, 3], [4, 5, 6, 7]]

    with tile.TileContext(nc) as tc:
        c.tc = tc
        with (
            tc.tile_pool(name="wp", bufs=1) as wp,      # streamed per-set weights
            tc.tile_pool(name="gp", bufs=1) as gp,      # persistent globals + activations
            tc.tile_pool(name="sp", bufs=3) as sp,      # scan transients
            tc.tile_pool(name="hp", bufs=4) as hp,      # staging
            tc.tile_pool(name="pp", bufs=4, space="PSUM") as pp,
            tc.tile_pool(name="yp", bufs=1, space="PSUM") as yp,
            tc.tile_pool(name="dp", bufs=1, space="DRAM") as dp,
        ):
            c.wp, c.gp, c.sp, c.hp, c.pp, c.yp, c.dp = wp, gp, sp, hp, pp, yp, dp

            # globals
            G = {}
            for nm in ('x_rhs', 'patch_lhsT', 'patch_b', 'patch_lhsT_sh',
                       'patch_b_sh', 'lm_lhsT'):
                t = din[nm]
                gt = gp.tile(list(t.shape), t.dtype, tag=f"g_{nm}")
                nc.sync.dma_start(gt[:], t[:])
                G[nm] = gt
            c.G = G
            ident = gp.tile([P, P], BF16, tag="ident")
            make_identity(nc, ident[:])
            c.ident = ident

            # persistent activations
            c.hbf = gp.tile([P, KT, L], BF16, tag="hbf")        # full h, bf16
            c.hms = gp.tile([P, L], F32, tag="hms")             # own DIM-shard master
            c.xi = gp.tile([P, MT_FULL, L + 6], BF16, tag="xi")
            c.sz = gp.tile([P, MT_SH, L], BF16, tag="sz")
            c.xc = gp.tile([P, MT_FULL, L], BF16, tag="xc")
            c.dblS = gp.tile([R + 2 * NS, L], BF16, tag="dblS")
            c.delta = gp.tile([P, MT_SH, L], BF16, tag="delta")
            c.u = gp.tile([P, MT_SH, L], BF16, tag="u")
            c.y2sum = gp.tile([P, MT_SH, L], BF16, tag="y2sum")
            c.state = gp.tile([P, MT_SH, NS], F32, tag="state")

            # zero the conv pads once
            for mt in range(MT_FULL):
                nc.gpsimd.memset(c.xi[:, mt, 0:3], 0.0)
                nc.gpsimd.memset(c.xi[:, mt, 3 + L:], 0.0)

            # DRAM staging for collectives (per half)
            c.bc_dram = [dp.tile([2 * NS, HL], BF16, tag=f"bc_dram{h}", name=f"bc_dram{h}")
                         for h in range(2)]
            c.up_in = [dp.tile([DIM, HL], BF16, tag=f"up_in{h}", name=f"up_in{h}")
                       for h in range(2)]
            c.up_rs = [dp.tile([P, HL], BF16, tag=f"up_rs{h}", name=f"up_rs{h}")
                       for h in range(2)]
            c.hag_in = [dp.tile([P, HL], BF16, tag=f"hag_in{h}", name=f"hag_in{h}")
                        for h in range(2)]
            c.hag_out = [dp.tile([DIM, HL], BF16, tag=f"hag_out{h}", name=f"hag_out{h}")
                         for h in range(2)]

            if repeat == 1:
                build_body(c)
            else:
                with tc.For_i(0, repeat, 1):
                    build_body(c)
    nc.compile()
    return nc


def load_set_weights(c, s):
    nc = c.nc
    W = {}
    for suff in ('wxi', 'wz', 'conv', 'convb', 'xp', 'dt', 'dtb', 'A', 'D', 'op'):
        t = c.din[f'{s}_{suff}']
        wt = c.wp.tile(list(t.shape), t.dtype, tag=f"w_{suff}")
        nc.sync.dma_start(wt[:], t[:])
        W[suff] = wt
    return W


def build_body(c):
    nc = c.nc
    L = c.L
    G = c.G

    # ---- patch embedding: full h bf16 + own fp32 shard ----
    for m in range(KT):
        for j in range(L // NT):
            ps = c.pp.tile([P, NT], F32, tag="ps")
            nc.tensor.matmul(ps[:], G['patch_lhsT'][:, m, :],
                             G['x_rhs'][:, bass.ts(j, NT)], start=True, stop=True)
            nc.scalar.activation(c.hbf[:, m, bass.ts(j, NT)], ps[:], AF.Identity,
                                 bias=G['patch_b'][:, m, :])
    for j in range(L // NT):
        ps = c.pp.tile([P, NT], F32, tag="ps")
        nc.tensor.matmul(ps[:], G['patch_lhsT_sh'][:],
                         G['x_rhs'][:, bass.ts(j, NT)], start=True, stop=True)
        nc.scalar.activation(c.hms[:, bass.ts(j, NT)], ps[:], AF.Identity,
                             bias=G['patch_b_sh'][:])

    # ---- blocks ----
    run_block(c, 'in', pair=True)
    for i in range(DEPTH):
        run_block(c, f'l{i}', pair=False)
    run_block(c, 'out', pair=True)

    # ---- lm head ----
    for j in range(L // NT):
        ps = c.pp.tile([P, NT], F32, tag="ps")
        for k in range(KT):
            nc.tensor.matmul(ps[:], G['lm_lhsT'][:, k, :],
                             c.hbf[:, k, bass.ts(j, NT)],
                             start=(k == 0), stop=(k == KT - 1))
        ot = c.hp.tile([P, NT], F32, tag="lmout", bufs=1)
        nc.vector.tensor_copy(ot[:], ps[:])
        nc.sync.dma_start(c.out_t[:, bass.ts(j, NT)], ot[:])


def prep_half(c, W, h, rev):
    """conv + xproj + delta + u for token half h (in_proj already done)."""
    nc = c.nc
    HL, jh = c.HL, c.jh
    t0 = h * HL

    # conv (full channels) -> silu -> xc
    for mt in range(MT_FULL):
        for j in range(jh):
            ps = c.pp.tile([P, NT], F32, tag="ps")
            for k in range(KC):
                off = (6 - k) if rev else k
                nc.tensor.matmul(ps[:], W['conv'][:, k, mt, :],
                                 c.xi[:, mt, t0 + j * NT + off: t0 + j * NT + off + NT],
                                 start=(k == 0), stop=(k == KC - 1))
            nc.scalar.activation(c.xc[:, mt, t0 + j * NT: t0 + (j + 1) * NT], ps[:],
                                 AF.Silu, bias=W['convb'][:, mt, :])

    # xproj (full, local): dbl[62, HL]
    for j in range(jh):
        psf = c.pp.tile([P, NT], F32, tag="ps")
        ps = psf[0:R + 2 * NS, :]
        for kt in range(MT_FULL):
            nc.tensor.matmul(ps, W['xp'][:, kt, :],
                             c.xc[:, kt, t0 + j * NT: t0 + (j + 1) * NT],
                             start=(kt == 0), stop=(kt == MT_FULL - 1))
        nc.vector.tensor_copy(c.dblS[:, t0 + j * NT: t0 + (j + 1) * NT], ps)

    nc.sync.dma_start(c.bc_dram[h][:], c.dblS[R:R + 2 * NS, t0:t0 + HL])

    # q = sigmoid(-(dtx+dtb)); delta tile holds lnq = -softplus(dtx+dtb)
    for mt in range(MT_SH):
        for j in range(jh):
            ps = c.pp.tile([P, NT], F32, tag="ps")
            nc.tensor.matmul(ps[:], W['dt'][:, mt, :],
                             c.dblS[0:R, t0 + j * NT: t0 + (j + 1) * NT],
                             start=True, stop=True)
            nc.scalar.activation(c.u[:, mt, t0 + j * NT: t0 + (j + 1) * NT],
                                 ps[:], AF.Sigmoid, bias=W['dtb'][:, mt, :],
                                 scale=-1.0)
    for mt in range(MT_SH):
        nc.scalar.activation(c.delta[:, mt, t0:t0 + HL], c.u[:, mt, t0:t0 + HL],
                             AF.Ln)
    # u = (-lnq) * xc_shard = softplus * xc
    for mt in range(MT_SH):
        nc.vector.scalar_tensor_tensor(c.u[:, mt, t0:t0 + HL],
                                       c.delta[:, mt, t0:t0 + HL], -1.0,
                                       c.xc[:, mt, t0:t0 + HL],
                                       op0=OP.mult, op1=OP.mult)


def scan_half(c, W, h, rev, first_half, di):
    """Selective scan + gating for token half h of direction di."""
    nc = c.nc
    HL, jh = c.HL, c.jh
    t0 = h * HL

    yps = [c.yp.tile([P, HL], F32, tag=f"yacc{mt}", name=f"yacc{mt}")
           for mt in range(MT_SH)]
    for n in range(NS):
        bcrep = c.sp.tile([P, 2, HL], BF16, tag="bcrep", bufs=4)
        nc.sync.dma_start(bcrep[:, None, :, :],
                          c.bc_dram[h][2 * n:2 * n + 2, :].partition_broadcast(P))
        brep = bcrep[:, 0, :]
        crep = bcrep[:, 1, :]
        bgeng = nc.gpsimd if (n % 3 == 1) else nc.vector
        for mt in range(MT_SH):
            ag = c.sp.tile([P, HL], BF16, tag=f"ag{mt}", bufs=2)
            nc.scalar.activation(ag[:], c.delta[:, mt, t0:t0 + HL], AF.Exp,
                                 scale=W['A'][:, mt, n:n + 1])

            bg = c.sp.tile([P, HL], BF16, tag=f"bg{mt}", bufs=2)
            bgeng.tensor_mul(bg[:], c.u[:, mt, t0:t0 + HL], brep)

            hg = c.sp.tile([P, HL], BF16, tag=f"hg{mt}", bufs=2)
            init = 0.0 if first_half else c.state[:, mt, n:n + 1]
            eng = nc.vector
            if rev:
                eng.tensor_tensor_scan(hg[:, ::-1], ag[:, ::-1], bg[:, ::-1],
                                       init, OP.mult, OP.add)
                if first_half:
                    nc.vector.tensor_copy(c.state[:, mt, n:n + 1], hg[:, 0:1])
            else:
                eng.tensor_tensor_scan(hg[:], ag[:], bg[:], init, OP.mult, OP.add)
                if first_half:
                    nc.vector.tensor_copy(c.state[:, mt, n:n + 1], hg[:, HL - 1:HL])

            hgc = c.sp.tile([P, HL], BF16, tag=f"bg{mt}", bufs=2, name=f"hgc{mt}")
            bgeng.tensor_mul(hgc[:], hg[:], crep)
            for ch in range(jh):
                nc.tensor.matmul(yps[mt][:, bass.ts(ch, NT)], c.ident[:],
                                 hgc[:, bass.ts(ch, NT)],
                                 start=(n == 0), stop=(n == NS - 1))

    # gating: y2 = yacc + D*xc ; y2s = y2 * sz (accumulate over directions)
    for mt in range(MT_SH):
        y2 = c.hp.tile([P, HL], BF16, tag="y2", bufs=1)
        nc.vector.scalar_tensor_tensor(y2[:], c.xc[:, mt, t0:t0 + HL],
                                       W['D'][:, mt, :], yps[mt][:],
                                       op0=OP.mult, op1=OP.add)
        if di == 0:
            nc.vector.tensor_mul(c.y2sum[:, mt, t0:t0 + HL], y2[:],
                                 c.sz[:, mt, t0:t0 + HL])
        else:
            y3 = c.hp.tile([P, HL], BF16, tag="y3", bufs=1)
            nc.gpsimd.tensor_mul(y3[:], y2[:], c.sz[:, mt, t0:t0 + HL])
            nc.vector.tensor_add(c.y2sum[:, mt, t0:t0 + HL],
                                 c.y2sum[:, mt, t0:t0 + HL], y3[:])


def residual_update(c, W, h):
    """out_proj partials for half h -> RS -> h master update -> AG -> hbf."""
    nc = c.nc
    HL, jh = c.HL, c.jh
    t0 = h * HL
    for m in range(KT):
        st = c.hp.tile([P, HL], BF16, tag="opst", bufs=2)
        for j in range(jh):
            ps = c.pp.tile([P, NT], F32, tag="ps")
            for kt in range(MT_SH):
                nc.tensor.matmul(ps[:], W['op'][:, kt, m, :],
                                 c.y2sum[:, kt, t0 + j * NT: t0 + (j + 1) * NT],
                                 start=(kt == 0), stop=(kt == MT_SH - 1))
            nc.scalar.activation(st[:, bass.ts(j, NT)], ps[:], AF.Copy)
        nc.sync.dma_start(c.up_in[h][m * P:(m + 1) * P, :], st[:])

    nc.gpsimd.collective_compute(
        "ReduceScatter", OP.add, replica_groups=c.groups,
        ins=[c.up_in[h][:].opt()], outs=[c.up_rs[h][:].opt()])

    ust = c.hp.tile([P, HL], BF16, tag="ust", bufs=1)
    nc.sync.dma_start(ust[:], c.up_rs[h][:])
    hst = c.hp.tile([P, HL], BF16, tag="hst", bufs=1)
    nc.vector.tensor_add(hst[:], c.hms[:, t0:t0 + HL], ust[:])
    nc.sync.dma_start(c.hag_in[h][:], hst[:])
    nc.gpsimd.tensor_add(c.hms[:, t0:t0 + HL], c.hms[:, t0:t0 + HL], ust[:])

    nc.gpsimd.collective_compute(
        "AllGather", OP.bypass, replica_groups=c.groups,
        ins=[c.hag_in[h][:].opt()], outs=[c.hag_out[h][:].opt()])
    nc.gpsimd.dma_start(c.hbf[:, :, t0:t0 + HL],
                        c.hag_out[h][:].rearrange("(k m) t -> m k t", k=KT))


def dirs_of(pair):
    return [(False, 0), (True, 1)] if pair else [(False, 0)]


def in_proj_half(c, W, h):
    nc = c.nc
    HL, jh = c.HL, c.jh
    for j in range(h * jh, (h + 1) * jh):
        for mt in range(MT_FULL):
            ps = c.pp.tile([P, NT], F32, tag="ps")
            for k in range(KT):
                nc.tensor.matmul(ps[:], W['wxi'][:, k, mt, :],
                                 c.hbf[:, k, bass.ts(j, NT)],
                                 start=(k == 0), stop=(k == KT - 1))
            dst = c.xi[:, mt, 3 + j * NT: 3 + (j + 1) * NT]
            if mt % 2 == 0:
                nc.vector.tensor_copy(dst, ps[:])
            else:
                nc.scalar.activation(dst, ps[:], AF.Copy)
        for mt in range(MT_SH):
            ps = c.pp.tile([P, NT], F32, tag="ps")
            for k in range(KT):
                nc.tensor.matmul(ps[:], W['wz'][:, k, mt, :],
                                 c.hbf[:, k, bass.ts(j, NT)],
                                 start=(k == 0), stop=(k == KT - 1))
            nc.scalar.activation(c.sz[:, mt, bass.ts(j, NT)], ps[:], AF.Silu)


def run_block(c, s, pair):
    nc = c.nc
    L = c.L
    W = load_set_weights(c, s)

    for rev, di in dirs_of(pair):
        if not rev:
            # forward direction: interleave in_proj per half with scans
            for h in (0, 1):
                in_proj_half(c, W, h)
                prep_half(c, W, h, rev)
                scan_half(c, W, h, rev, h == 0, di)
                if not pair:
                    residual_update(c, W, h)
        else:
            # reverse direction of a pair: xi already complete; h1 first
            prep_half(c, W, 1, rev)
            scan_half(c, W, 1, rev, True, di)
            residual_update(c, W, 1)
            prep_half(c, W, 0, rev)
            scan_half(c, W, 0, rev, False, di)
            residual_update(c, W, 0)


_KERNEL_CACHE = {}


def get_kernel(L, repeat=1):
    key = (L, repeat)
    if key not in _KERNEL_CACHE:
        _KERNEL_CACHE[key] = build_kernel(L, repeat)
    return _KERNEL_CACHE[key]


def kernel(**inputs):
    L = int(np.asarray(inputs['x']).shape[1])
    nc = get_kernel(L)
    in_maps = [prep_core_inputs(cc, inputs, L) for cc in range(NCORES)]
    res = run_bass_kernel_spmd(nc, in_maps, list(range(NCORES)))
    outs = [np.asarray(res.results[cc]['out'], np.float32) for cc in range(NCORES)]
    full = []
    for srow in range(B):
        sm = np.concatenate(outs[srow * TPG:(srow + 1) * TPG], axis=0)  # (472, L)
        full.append(sm.T)
    return np.ascontiguousarray(np.stack(full, axis=0))


# revision 44
# speedup vs baseline: 1.9094x; 1.0586x over previous
"""Bass/Trainium2 kernel for nn_BysMamba (bidirectional + stacked Mamba LM).

Sharding: DP2 x TP4. Cores 0-3 own sample 0, cores 4-7 sample 1 (full
L=2048 sequence each). Within a sample group, d_inner ED=944 is split 4
ways (236 channels/core) for scan/gating/out_proj, while the x-branch of
in_proj, the causal conv and x_proj are computed redundantly on the full
944 channels so dbl/B/C/delta need no collective. The full-channel layout
is PERMUTED per core (own shard first) so the SPMD program is uniform.
Per layer the only collectives are a 4-way ReduceScatter of out_proj
partials and a 4-way AllGather of the updated residual (bf16), each split
in two token halves so they overlap the other half's scan compute. The
residual h lives in SBUF: fp32 master of this core's 118-row DIM shard +
full bf16 copy from the AllGather. exp(A_n*delta) exploits the S4D-real
structure (A_n ~ -(n+1)): low n direct on Act, high n chained multiply by
q = exp(-delta) on DVE. Scans split across DVE and Pool; y contracts over
the 16 states via identity matmuls accumulating in PSUM.
"""
import sys
sys.path.insert(0, '/opt/trn_rl_repo')

import numpy as np
import ml_dtypes

import concourse.bass as bass
from concourse import bacc
import concourse.mybir as mybir
import concourse.tile as tile
from concourse.masks import make_identity
from concourse.bass_utils import run_bass_kernel_spmd

F32 = mybir.dt.float32
BF16 = mybir.dt.bfloat16
AF = mybir.ActivationFunctionType
OP = mybir.AluOpType

V = 472
DIM = 472
ED = 944
NS = 16
KC = 4
R = 30
DEPTH = 8
B = 2

NCORES = 8
TPG = 4                      # tensor-parallel group size
P = 118                      # partition tile
KT = DIM // P                # 4 k-tiles over DIM
MT_FULL = ED // P            # 8 channel tiles (full)
EC = ED // TPG               # 236 channels per core
MT_SH = EC // P              # 2 channel tiles (shard)
NT = 512                     # psum column tile

ACT_N = 10                   # n < ACT_N: ag via Act exp; else DVE chain

SETS = ['in'] + [f'l{i}' for i in range(DEPTH)] + ['out']


def _bf(x):
    return np.ascontiguousarray(np.asarray(x, np.float32).astype(ml_dtypes.bfloat16))


def _f32(x):
    return np.ascontiguousarray(np.asarray(x, np.float32))


def prep_core_inputs(core, inputs, L):
    s, r = divmod(core, TPG)
    e0 = r * EC
    perm = np.r_[e0:e0 + EC, 0:e0, e0 + EC:ED]         # own shard first
    d = {}
    x = np.asarray(inputs['x'], np.float32)[s]         # (L, 3, 3)
    d['x_rhs'] = _bf(x.reshape(L, 9).T)                # (9, L)
    pw = np.asarray(inputs['patch_w'], np.float32)[:, 0].reshape(V, 9)
    d['patch_lhsT'] = _bf(pw.T.reshape(9, KT, P))      # (9, 4, 118)
    d['patch_b'] = _f32(np.asarray(inputs['patch_b']).reshape(KT, P, 1).transpose(1, 0, 2))
    d['patch_lhsT_sh'] = _bf(pw.T[:, r * P:(r + 1) * P])          # (9, 118)
    d['patch_b_sh'] = _f32(np.asarray(inputs['patch_b'])[r * P:(r + 1) * P].reshape(P, 1))
    lm = np.asarray(inputs['lm_head_w'], np.float32)[r * P:(r + 1) * P]   # (118, 472)
    d['lm_lhsT'] = _bf(lm.T.reshape(KT, P, P).transpose(1, 0, 2))         # (118,4,118)
    for snm in SETS:
        if snm == 'in':
            g = lambda n: np.asarray(inputs[f'in_{n}'], np.float32)
        elif snm == 'out':
            g = lambda n: np.asarray(inputs[f'out_{n}'], np.float32)
        else:
            li = int(snm[1:])
            g = lambda n, li=li: np.asarray(inputs[f'lay_{n}'], np.float32)[li]
        ip = g('inproj_w')
        # xi part: FULL 944 rows, permuted; lhsT (472,944)->(118,4,8,118)
        wxi = ip[:ED][perm].T.reshape(KT, P, MT_FULL, P).transpose(1, 0, 2, 3)
        d[f'{snm}_wxi'] = _bf(wxi)
        wz = ip[ED + e0:ED + e0 + EC].T.reshape(KT, P, MT_SH, P).transpose(1, 0, 2, 3)
        d[f'{snm}_wz'] = _bf(wz)
        cw = g('conv_w')[:, 0][perm]                    # (944, 4) permuted
        diag = np.zeros((KC, MT_FULL, P, P), np.float32)
        idx = np.arange(P)
        for k in range(KC):
            for mt in range(MT_FULL):
                diag[k, mt, idx, idx] = cw[mt * P:(mt + 1) * P, k]
        d[f'{snm}_conv'] = _bf(diag.transpose(2, 0, 1, 3))   # (118,4,8,118)
        d[f'{snm}_convb'] = _f32(g('conv_b')[perm].reshape(MT_FULL, P, 1).transpose(1, 0, 2))
        xpw = g('xproj_w')[:, perm]                     # (62, 944)
        rowp = np.r_[0:R, [R + i // 2 + NS * (i % 2) for i in range(2 * NS)]]
        xpw = xpw[rowp]                                 # B/C interleaved pairs
        d[f'{snm}_xp'] = _bf(xpw.T.reshape(MT_FULL, P, R + 2 * NS).transpose(1, 0, 2))
        d[f'{snm}_dt'] = _bf(g('dt_w')[e0:e0 + EC].T.reshape(R, MT_SH, P))  # (30,2,118)
        d[f'{snm}_dtb'] = _f32(-g('dt_b')[e0:e0 + EC].reshape(MT_SH, P, 1).transpose(1, 0, 2))
        d[f'{snm}_A'] = _f32(np.exp(g('Alog')[e0:e0 + EC]).reshape(MT_SH, P, NS).transpose(1, 0, 2))
        d[f'{snm}_D'] = _f32(g('D')[e0:e0 + EC].reshape(MT_SH, P, 1).transpose(1, 0, 2))
        op = g('outproj_w')[:, e0:e0 + EC].T            # (236, 472)
        d[f'{snm}_op'] = _bf(op.reshape(MT_SH, P, KT, P).transpose(1, 0, 2, 3))
    return d


class Ctx:
    pass


def build_kernel(L, repeat=1):
    HL = L // 2                  # half length
    jh = HL // NT                # 512-tiles per half

    nc = bacc.Bacc(num_devices=NCORES)
    din = {}

    def dram_in(name, shape, dt):
        din[name] = nc.dram_tensor(name, list(shape), dt, kind="ExternalInput")

    dram_in('x_rhs', (9, L), BF16)
    dram_in('patch_lhsT', (9, KT, P), BF16)
    dram_in('patch_b', (P, KT, 1), F32)
    dram_in('patch_lhsT_sh', (9, P), BF16)
    dram_in('patch_b_sh', (P, 1), F32)
    dram_in('lm_lhsT', (P, KT, P), BF16)
    for s in SETS:
        dram_in(f'{s}_wxi', (P, KT, MT_FULL, P), BF16)
        dram_in(f'{s}_wz', (P, KT, MT_SH, P), BF16)
        dram_in(f'{s}_conv', (P, KC, MT_FULL, P), BF16)
        dram_in(f'{s}_convb', (P, MT_FULL, 1), F32)
        dram_in(f'{s}_xp', (P, MT_FULL, R + 2 * NS), BF16)
        dram_in(f'{s}_dt', (R, MT_SH, P), BF16)
        dram_in(f'{s}_dtb', (P, MT_SH, 1), F32)
        dram_in(f'{s}_A', (P, MT_SH, NS), F32)
        dram_in(f'{s}_D', (P, MT_SH, 1), F32)
        dram_in(f'{s}_op', (P, MT_SH, KT, P), BF16)
    out_t = nc.dram_tensor('out', [P, L], F32, kind="ExternalOutput")

    c = Ctx()
    c.nc, c.din, c.out_t = nc, din, out_t
    c.L, c.HL, c.jh = L, HL, jh
    c.pending = []
    c.groups = [[0, 1, 2, 3], [4, 5, 6, 7]]

    with tile.TileContext(nc) as tc:
        c.tc = tc
        with (
            tc.tile_pool(name="wp", bufs=1) as wp,      # streamed per-set weights
            tc.tile_pool(name="gp", bufs=1) as gp,      # persistent globals + activations
            tc.tile_pool(name="sp", bufs=3) as sp,      # scan transients
            tc.tile_pool(name="hp", bufs=4) as hp,      # staging
            tc.tile_pool(name="pp", bufs=4, space="PSUM") as pp,
            tc.tile_pool(name="yp", bufs=1, space="PSUM") as yp,
            tc.tile_pool(name="dp", bufs=1, space="DRAM") as dp,
        ):
            c.wp, c.gp, c.sp, c.hp, c.pp, c.yp, c.dp = wp, gp, sp, hp, pp, yp, dp

            # globals
            G = {}
            for nm in ('x_rhs', 'patch_lhsT', 'patch_b', 'patch_lhsT_sh',
                       'patch_b_sh', 'lm_lhsT'):
                t = din[nm]
                gt = gp.tile(list(t.shape), t.dtype, tag=f"g_{nm}")
                nc.sync.dma_start(gt[:], t[:])
                G[nm] = gt
            c.G = G
            ident = gp.tile([P, P], BF16, tag="ident")
            make_identity(nc, ident[:])
            c.ident = ident

            # persistent activations
            c.hbf = gp.tile([P, KT, L], BF16, tag="hbf")        # full h, bf16
            c.xi = gp.tile([P, MT_FULL, L + 6], BF16, tag="xi")
            c.sz = gp.tile([P, MT_SH, L], BF16, tag="sz")
            c.xc = gp.tile([P, MT_FULL, L], BF16, tag="xc")
            c.dblS = gp.tile([R + 2 * NS, L], BF16, tag="dblS")
            c.delta = gp.tile([P, MT_SH, L], BF16, tag="delta")
            c.u = gp.tile([P, MT_SH, L], BF16, tag="u")
            c.y2sum = gp.tile([P, MT_SH, L], BF16, tag="y2sum")
            c.state = gp.tile([P, MT_SH, NS], F32, tag="state")

            # zero the conv pads once
            for mt in range(MT_FULL):
                nc.gpsimd.memset(c.xi[:, mt, 0:3], 0.0)
                nc.gpsimd.memset(c.xi[:, mt, 3 + L:], 0.0)

            # DRAM staging for collectives (per half)
            c.bc_dram = [dp.tile([2 * NS, HL], BF16, tag=f"bc_dram{h}", name=f"bc_dram{h}")
                         for h in range(2)]
            c.up_in = [dp.tile([DIM, HL], BF16, tag=f"up_in{h}", name=f"up_in{h}")
                       for h in range(2)]
            c.up_rs = [dp.tile([P, HL], BF16, tag=f"up_rs{h}", name=f"up_rs{h}")
                       for h in range(2)]
            c.hag_out = [dp.tile([DIM, HL], BF16, tag=f"hag_out{h}", name=f"hag_out{h}")
                         for h in range(2)]

            if repeat == 1:
                build_body(c)
            else:
                with tc.For_i(0, repeat, 1):
                    build_body(c)
    nc.compile()
    return nc


def load_set_weights(c, s):
    nc = c.nc
    W = {}
    for suff in ('wxi', 'wz', 'conv', 'convb', 'xp', 'dt', 'dtb', 'A', 'D', 'op'):
        t = c.din[f'{s}_{suff}']
        wt = c.wp.tile(list(t.shape), t.dtype, tag=f"w_{suff}")
        nc.sync.dma_start(wt[:], t[:])
        W[suff] = wt
    return W


def build_body(c):
    nc = c.nc
    L = c.L
    G = c.G

    # ---- patch embedding: full h bf16 + own fp32 shard ----
    for m in range(KT):
        for j in range(L // NT):
            ps = c.pp.tile([P, NT], F32, tag="ps")
            nc.tensor.matmul(ps[:], G['patch_lhsT'][:, m, :],
                             G['x_rhs'][:, bass.ts(j, NT)], start=True, stop=True)
            nc.scalar.activation(c.hbf[:, m, bass.ts(j, NT)], ps[:], AF.Identity,
                                 bias=G['patch_b'][:, m, :])
    # ---- blocks ----
    run_block(c, 'in', pair=True)
    for i in range(DEPTH):
        run_block(c, f'l{i}', pair=False)
    run_block(c, 'out', pair=True)

    # ---- lm head: per half, right after that half's residual apply ----
    jh = c.jh
    for ph in c.pending:
        residual_apply(c, ph)
        for j in range(ph * jh, (ph + 1) * jh):
            ps = c.pp.tile([P, NT], F32, tag="ps")
            for k in range(KT):
                nc.tensor.matmul(ps[:], G['lm_lhsT'][:, k, :],
                                 c.hbf[:, k, bass.ts(j, NT)],
                                 start=(k == 0), stop=(k == KT - 1))
            ot = c.hp.tile([P, NT], F32, tag="lmout", bufs=1)
            nc.vector.tensor_copy(ot[:], ps[:])
            nc.sync.dma_start(c.out_t[:, bass.ts(j, NT)], ot[:])
    c.pending = []


def prep_half(c, W, h, rev):
    """conv + xproj + delta + u for token half h (in_proj already done)."""
    nc = c.nc
    HL, jh = c.HL, c.jh
    t0 = h * HL

    # conv (full channels) -> silu -> xc
    for mt in range(MT_FULL):
        for j in range(jh):
            ps = c.pp.tile([P, NT], F32, tag="ps")
            for k in range(KC):
                off = (6 - k) if rev else k
                nc.tensor.matmul(ps[:], W['conv'][:, k, mt, :],
                                 c.xi[:, mt, t0 + j * NT + off: t0 + j * NT + off + NT],
                                 start=(k == 0), stop=(k == KC - 1))
            nc.scalar.activation(c.xc[:, mt, t0 + j * NT: t0 + (j + 1) * NT], ps[:],
                                 AF.Silu, bias=W['convb'][:, mt, :])

    # xproj (full, local): dbl[62, HL]
    for j in range(jh):
        psf = c.pp.tile([P, NT], F32, tag="ps")
        ps = psf[0:R + 2 * NS, :]
        for kt in range(MT_FULL):
            nc.tensor.matmul(ps, W['xp'][:, kt, :],
                             c.xc[:, kt, t0 + j * NT: t0 + (j + 1) * NT],
                             start=(kt == 0), stop=(kt == MT_FULL - 1))
        nc.vector.tensor_copy(c.dblS[:, t0 + j * NT: t0 + (j + 1) * NT], ps)

    nc.sync.dma_start(c.bc_dram[h][:], c.dblS[R:R + 2 * NS, t0:t0 + HL])

    # q = sigmoid(-(dtx+dtb)); delta tile holds lnq = -softplus(dtx+dtb)
    for mt in range(MT_SH):
        for j in range(jh):
            ps = c.pp.tile([P, NT], F32, tag="ps")
            nc.tensor.matmul(ps[:], W['dt'][:, mt, :],
                             c.dblS[0:R, t0 + j * NT: t0 + (j + 1) * NT],
                             start=True, stop=True)
            nc.scalar.activation(c.u[:, mt, t0 + j * NT: t0 + (j + 1) * NT],
                                 ps[:], AF.Sigmoid, bias=W['dtb'][:, mt, :],
                                 scale=-1.0)
    for mt in range(MT_SH):
        nc.scalar.activation(c.delta[:, mt, t0:t0 + HL], c.u[:, mt, t0:t0 + HL],
                             AF.Ln)
    # u = (-lnq) * xc_shard = softplus * xc
    for mt in range(MT_SH):
        nc.vector.scalar_tensor_tensor(c.u[:, mt, t0:t0 + HL],
                                       c.delta[:, mt, t0:t0 + HL], -1.0,
                                       c.xc[:, mt, t0:t0 + HL],
                                       op0=OP.mult, op1=OP.mult)


def scan_half(c, W, h, rev, first_half, di):
    """Selective scan + gating for token half h of direction di."""
    nc = c.nc
    HL, jh = c.HL, c.jh
    t0 = h * HL

    yps = [c.yp.tile([P, HL], F32, tag=f"yacc{mt}", name=f"yacc{mt}")
           for mt in range(MT_SH)]
    for n in range(NS):
        bcrep = c.sp.tile([P, 2, HL], BF16, tag="bcrep", bufs=4)
        nc.sync.dma_start(bcrep[:, None, :, :],
                          c.bc_dram[h][2 * n:2 * n + 2, :].partition_broadcast(P))
        brep = bcrep[:, 0, :]
        crep = bcrep[:, 1, :]
        bgeng = nc.gpsimd if (n % 3 == 1) else nc.vector
        hgeng = bgeng
        for mt in range(MT_SH):
            ag = c.sp.tile([P, HL], BF16, tag=f"ag{mt}", bufs=2)
            nc.scalar.activation(ag[:], c.delta[:, mt, t0:t0 + HL], AF.Exp,
                                 scale=W['A'][:, mt, n:n + 1])

            bg = c.sp.tile([P, HL], BF16, tag=f"bg{mt}", bufs=3)
            bgeng.tensor_mul(bg[:], c.u[:, mt, t0:t0 + HL], brep)

            hg = c.sp.tile([P, HL], BF16, tag=f"hg{mt}", bufs=3)
            init = 0.0 if first_half else c.state[:, mt, n:n + 1]
            eng = nc.vector
            if rev:
                eng.tensor_tensor_scan(hg[:, ::-1], ag[:, ::-1], bg[:, ::-1],
                                       init, OP.mult, OP.add)
                if first_half:
                    nc.vector.tensor_copy(c.state[:, mt, n:n + 1], hg[:, 0:1])
            else:
                eng.tensor_tensor_scan(hg[:], ag[:], bg[:], init, OP.mult, OP.add)
                if first_half:
                    nc.vector.tensor_copy(c.state[:, mt, n:n + 1], hg[:, HL - 1:HL])

            hgc = c.sp.tile([P, HL], BF16, tag=f"bg{mt}", bufs=3, name=f"hgc{mt}")
            hgeng.tensor_mul(hgc[:], hg[:], crep)
            for ch in range(jh):
                nc.tensor.matmul(yps[mt][:, bass.ts(ch, NT)], c.ident[:],
                                 hgc[:, bass.ts(ch, NT)],
                                 start=(n == 0), stop=(n == NS - 1))

    # gating: y2 = yacc + D*xc ; y2s = y2 * sz (accumulate over directions)
    for mt in range(MT_SH):
        y2 = c.hp.tile([P, HL], BF16, tag="y2", bufs=1)
        nc.vector.scalar_tensor_tensor(y2[:], c.xc[:, mt, t0:t0 + HL],
                                       W['D'][:, mt, :], yps[mt][:],
                                       op0=OP.mult, op1=OP.add)
        if di == 0:
            nc.vector.tensor_mul(c.y2sum[:, mt, t0:t0 + HL], y2[:],
                                 c.sz[:, mt, t0:t0 + HL])
        else:
            y3 = c.hp.tile([P, HL], BF16, tag="y3", bufs=1)
            nc.gpsimd.tensor_mul(y3[:], y2[:], c.sz[:, mt, t0:t0 + HL])
            nc.vector.tensor_add(c.y2sum[:, mt, t0:t0 + HL],
                                 c.y2sum[:, mt, t0:t0 + HL], y3[:])


def residual_update(c, W, h):
    """out_proj partials for half h -> RS -> AG of the delta (no staging)."""
    nc = c.nc
    HL, jh = c.HL, c.jh
    t0 = h * HL
    for m in range(KT):
        st = c.hp.tile([P, HL], BF16, tag="opst", bufs=2)
        for j in range(jh):
            ps = c.pp.tile([P, NT], F32, tag="ps")
            for kt in range(MT_SH):
                nc.tensor.matmul(ps[:], W['op'][:, kt, m, :],
                                 c.y2sum[:, kt, t0 + j * NT: t0 + (j + 1) * NT],
                                 start=(kt == 0), stop=(kt == MT_SH - 1))
            nc.scalar.activation(st[:, bass.ts(j, NT)], ps[:], AF.Copy)
        nc.sync.dma_start(c.up_in[h][m * P:(m + 1) * P, :], st[:])

    nc.gpsimd.collective_compute(
        "ReduceScatter", OP.add, replica_groups=c.groups,
        ins=[c.up_in[h][:].opt()], outs=[c.up_rs[h][:].opt()])
    nc.gpsimd.collective_compute(
        "AllGather", OP.bypass, replica_groups=c.groups,
        ins=[c.up_rs[h][:].opt()], outs=[c.hag_out[h][:].opt()])


def residual_apply(c, h):
    """Load the gathered residual delta and add it into hbf."""
    nc = c.nc
    HL = c.HL
    t0 = h * HL
    for k in range(KT):
        upf = c.sp.tile([P, HL], BF16, tag="upf", bufs=2, name=f"upf{k}")
        nc.sync.dma_start(upf[:], c.hag_out[h][k * P:(k + 1) * P, :])
        nc.vector.tensor_add(c.hbf[:, k, t0:t0 + HL], c.hbf[:, k, t0:t0 + HL],
                             upf[:])


def dirs_of(pair):
    return [(False, 0), (True, 1)] if pair else [(False, 0)]


def in_proj_half(c, W, h):
    nc = c.nc
    HL, jh = c.HL, c.jh
    for j in range(h * jh, (h + 1) * jh):
        for mt in range(MT_FULL):
            ps = c.pp.tile([P, NT], F32, tag="ps")
            for k in range(KT):
                nc.tensor.matmul(ps[:], W['wxi'][:, k, mt, :],
                                 c.hbf[:, k, bass.ts(j, NT)],
                                 start=(k == 0), stop=(k == KT - 1))
            dst = c.xi[:, mt, 3 + j * NT: 3 + (j + 1) * NT]
            nc.scalar.activation(dst, ps[:], AF.Copy)
        for mt in range(MT_SH):
            ps = c.pp.tile([P, NT], F32, tag="ps")
            for k in range(KT):
                nc.tensor.matmul(ps[:], W['wz'][:, k, mt, :],
                                 c.hbf[:, k, bass.ts(j, NT)],
                                 start=(k == 0), stop=(k == KT - 1))
            nc.scalar.activation(c.sz[:, mt, bass.ts(j, NT)], ps[:], AF.Silu)


def run_block(c, s, pair):
    nc = c.nc
    L = c.L
    for ph in c.pending:
        residual_apply(c, ph)
    c.pending = []
    W = load_set_weights(c, s)

    for rev, di in dirs_of(pair):
        if not rev:
            for h in (0, 1):
                in_proj_half(c, W, h)
                prep_half(c, W, h, rev)
                scan_half(c, W, h, rev, h == 0, di)
                if not pair:
                    residual_update(c, W, h)
            if not pair:
                c.pending = [0, 1]
        else:
            # reverse direction of a pair: xi already complete; h1 first
            prep_half(c, W, 1, rev)
            scan_half(c, W, 1, rev, True, di)
            residual_update(c, W, 1)
            prep_half(c, W, 0, rev)
            scan_half(c, W, 0, rev, False, di)
            residual_update(c, W, 0)
            c.pending = [1, 0]


_KERNEL_CACHE = {}


def get_kernel(L, repeat=1):
    key = (L, repeat)
    if key not in _KERNEL_CACHE:
        _KERNEL_CACHE[key] = build_kernel(L, repeat)
    return _KERNEL_CACHE[key]


def kernel(**inputs):
    L = int(np.asarray(inputs['x']).shape[1])
    nc = get_kernel(L)
    in_maps = [prep_core_inputs(cc, inputs, L) for cc in range(NCORES)]
    res = run_bass_kernel_spmd(nc, in_maps, list(range(NCORES)))
    outs = [np.asarray(res.results[cc]['out'], np.float32) for cc in range(NCORES)]
    full = []
    for srow in range(B):
        sm = np.concatenate(outs[srow * TPG:(srow + 1) * TPG], axis=0)  # (472, L)
        full.append(sm.T)
    return np.ascontiguousarray(np.stack(full, axis=0))


# revision 48
# speedup vs baseline: 1.9207x; 1.0059x over previous
"""Bass/Trainium2 kernel for nn_BysMamba (bidirectional + stacked Mamba LM).

Sharding: DP2 x TP4. Cores 0-3 own sample 0, cores 4-7 sample 1 (full
L=2048 sequence each). Within a sample group, d_inner ED=944 is split 4
ways (236 channels/core) for scan/gating/out_proj, while the x-branch of
in_proj, the causal conv and x_proj are computed redundantly on the full
944 channels so dbl/B/C/delta need no collective. The full-channel layout
is PERMUTED per core (own shard first) so the SPMD program is uniform.
Per layer the only collectives are a 4-way ReduceScatter of out_proj
partials and a 4-way AllGather of the reduced delta, issued back-to-back
(no staging between) and split in two token halves so they overlap the
other half's scan compute. The residual h lives in SBUF as bf16 only; the
gathered delta is DMA-loaded and added into it, deferred to the next
block so the waiting loads never head-of-line block an engine queue.
delta path: q = sigmoid(-(dt_x+dt_b)) on Act, lnq = Ln(q), u =
(-lnq)*xc via one DVE stt, and every decay exp(A_n*delta) is a single Act
Exp of lnq scaled by +exp(Alog)_n (exact, S4D-real structure). Scans run
on DVE (illegal on Pool); bg feeds the scan so it stays on DVE while hgc
splits to Pool; y contracts over the 16 states via identity matmuls
accumulating in PSUM banks of 512 fp32 cols.
"""
import sys
sys.path.insert(0, '/opt/trn_rl_repo')

import numpy as np
import ml_dtypes

import concourse.bass as bass
from concourse import bacc
import concourse.mybir as mybir
import concourse.tile as tile
from concourse.masks import make_identity
from concourse.bass_utils import run_bass_kernel_spmd

F32 = mybir.dt.float32
BF16 = mybir.dt.bfloat16
AF = mybir.ActivationFunctionType
OP = mybir.AluOpType

V = 472
DIM = 472
ED = 944
NS = 16
KC = 4
R = 30
DEPTH = 8
B = 2

NCORES = 8
TPG = 4                      # tensor-parallel group size
P = 118                      # partition tile
KT = DIM // P                # 4 k-tiles over DIM
MT_FULL = ED // P            # 8 channel tiles (full)
EC = ED // TPG               # 236 channels per core
MT_SH = EC // P              # 2 channel tiles (shard)
NT = 512                     # psum column tile

ACT_N = 10                   # n < ACT_N: ag via Act exp; else DVE chain

SETS = ['in'] + [f'l{i}' for i in range(DEPTH)] + ['out']


def _bf(x):
    return np.ascontiguousarray(np.asarray(x, np.float32).astype(ml_dtypes.bfloat16))


def _f32(x):
    return np.ascontiguousarray(np.asarray(x, np.float32))


def prep_core_inputs(core, inputs, L):
    s, r = divmod(core, TPG)
    e0 = r * EC
    perm = np.r_[e0:e0 + EC, 0:e0, e0 + EC:ED]         # own shard first
    d = {}
    x = np.asarray(inputs['x'], np.float32)[s]         # (L, 3, 3)
    d['x_rhs'] = _bf(x.reshape(L, 9).T)                # (9, L)
    pw = np.asarray(inputs['patch_w'], np.float32)[:, 0].reshape(V, 9)
    d['patch_lhsT'] = _bf(pw.T.reshape(9, KT, P))      # (9, 4, 118)
    d['patch_b'] = _f32(np.asarray(inputs['patch_b']).reshape(KT, P, 1).transpose(1, 0, 2))
    d['patch_lhsT_sh'] = _bf(pw.T[:, r * P:(r + 1) * P])          # (9, 118)
    d['patch_b_sh'] = _f32(np.asarray(inputs['patch_b'])[r * P:(r + 1) * P].reshape(P, 1))
    lm = np.asarray(inputs['lm_head_w'], np.float32)[r * P:(r + 1) * P]   # (118, 472)
    d['lm_lhsT'] = _bf(lm.T.reshape(KT, P, P).transpose(1, 0, 2))         # (118,4,118)
    for snm in SETS:
        if snm == 'in':
            g = lambda n: np.asarray(inputs[f'in_{n}'], np.float32)
        elif snm == 'out':
            g = lambda n: np.asarray(inputs[f'out_{n}'], np.float32)
        else:
            li = int(snm[1:])
            g = lambda n, li=li: np.asarray(inputs[f'lay_{n}'], np.float32)[li]
        ip = g('inproj_w')
        # xi part: FULL 944 rows, permuted; lhsT (472,944)->(118,4,8,118)
        wxi = ip[:ED][perm].T.reshape(KT, P, MT_FULL, P).transpose(1, 0, 2, 3)
        d[f'{snm}_wxi'] = _bf(wxi)
        wz = ip[ED + e0:ED + e0 + EC].T.reshape(KT, P, MT_SH, P).transpose(1, 0, 2, 3)
        d[f'{snm}_wz'] = _bf(wz)
        cw = g('conv_w')[:, 0][perm]                    # (944, 4) permuted
        diag = np.zeros((KC, MT_FULL, P, P), np.float32)
        idx = np.arange(P)
        for k in range(KC):
            for mt in range(MT_FULL):
                diag[k, mt, idx, idx] = cw[mt * P:(mt + 1) * P, k]
        d[f'{snm}_conv'] = _bf(diag.transpose(2, 0, 1, 3))   # (118,4,8,118)
        d[f'{snm}_convb'] = _f32(g('conv_b')[perm].reshape(MT_FULL, P, 1).transpose(1, 0, 2))
        xpw = g('xproj_w')[:, perm]                     # (62, 944)
        rowp = np.r_[0:R, [R + i // 2 + NS * (i % 2) for i in range(2 * NS)]]
        xpw = xpw[rowp]                                 # B/C interleaved pairs
        d[f'{snm}_xp'] = _bf(xpw.T.reshape(MT_FULL, P, R + 2 * NS).transpose(1, 0, 2))
        d[f'{snm}_dt'] = _bf(g('dt_w')[e0:e0 + EC].T.reshape(R, MT_SH, P))  # (30,2,118)
        d[f'{snm}_dtb'] = _f32(-g('dt_b')[e0:e0 + EC].reshape(MT_SH, P, 1).transpose(1, 0, 2))
        d[f'{snm}_A'] = _f32(np.exp(g('Alog')[e0:e0 + EC]).reshape(MT_SH, P, NS).transpose(1, 0, 2))
        d[f'{snm}_D'] = _f32(g('D')[e0:e0 + EC].reshape(MT_SH, P, 1).transpose(1, 0, 2))
        op = g('outproj_w')[:, e0:e0 + EC].T            # (236, 472)
        d[f'{snm}_op'] = _bf(op.reshape(MT_SH, P, KT, P).transpose(1, 0, 2, 3))
    return d


class Ctx:
    pass


def build_kernel(L, repeat=1):
    HL = L // 2                  # half length
    jh = HL // NT                # 512-tiles per half

    nc = bacc.Bacc(num_devices=NCORES)
    din = {}

    def dram_in(name, shape, dt):
        din[name] = nc.dram_tensor(name, list(shape), dt, kind="ExternalInput")

    dram_in('x_rhs', (9, L), BF16)
    dram_in('patch_lhsT', (9, KT, P), BF16)
    dram_in('patch_b', (P, KT, 1), F32)
    dram_in('patch_lhsT_sh', (9, P), BF16)
    dram_in('patch_b_sh', (P, 1), F32)
    dram_in('lm_lhsT', (P, KT, P), BF16)
    for s in SETS:
        dram_in(f'{s}_wxi', (P, KT, MT_FULL, P), BF16)
        dram_in(f'{s}_wz', (P, KT, MT_SH, P), BF16)
        dram_in(f'{s}_conv', (P, KC, MT_FULL, P), BF16)
        dram_in(f'{s}_convb', (P, MT_FULL, 1), F32)
        dram_in(f'{s}_xp', (P, MT_FULL, R + 2 * NS), BF16)
        dram_in(f'{s}_dt', (R, MT_SH, P), BF16)
        dram_in(f'{s}_dtb', (P, MT_SH, 1), F32)
        dram_in(f'{s}_A', (P, MT_SH, NS), F32)
        dram_in(f'{s}_D', (P, MT_SH, 1), F32)
        dram_in(f'{s}_op', (P, MT_SH, KT, P), BF16)
    out_t = nc.dram_tensor('out', [P, L], F32, kind="ExternalOutput")

    c = Ctx()
    c.nc, c.din, c.out_t = nc, din, out_t
    c.L, c.HL, c.jh = L, HL, jh
    c.pending = []
    c.groups = [[0, 1, 2, 3], [4, 5, 6, 7]]

    with tile.TileContext(nc) as tc:
        c.tc = tc
        with (
            tc.tile_pool(name="wp", bufs=1) as wp,      # streamed per-set weights
            tc.tile_pool(name="gp", bufs=1) as gp,      # persistent globals + activations
            tc.tile_pool(name="sp", bufs=3) as sp,      # scan transients
            tc.tile_pool(name="hp", bufs=4) as hp,      # staging
            tc.tile_pool(name="pp", bufs=4, space="PSUM") as pp,
            tc.tile_pool(name="yp", bufs=1, space="PSUM") as yp,
            tc.tile_pool(name="dp", bufs=1, space="DRAM") as dp,
        ):
            c.wp, c.gp, c.sp, c.hp, c.pp, c.yp, c.dp = wp, gp, sp, hp, pp, yp, dp

            # globals
            G = {}
            for nm in ('x_rhs', 'patch_lhsT', 'patch_b', 'patch_lhsT_sh',
                       'patch_b_sh', 'lm_lhsT'):
                t = din[nm]
                gt = gp.tile(list(t.shape), t.dtype, tag=f"g_{nm}")
                nc.sync.dma_start(gt[:], t[:])
                G[nm] = gt
            c.G = G
            ident = gp.tile([P, P], BF16, tag="ident")
            make_identity(nc, ident[:])
            c.ident = ident

            # persistent activations
            c.hbf = gp.tile([P, KT, L], BF16, tag="hbf")        # full h, bf16
            c.xi = gp.tile([P, MT_FULL, L + 6], BF16, tag="xi")
            c.sz = gp.tile([P, MT_SH, L], BF16, tag="sz")
            c.xc = gp.tile([P, MT_FULL, L], BF16, tag="xc")
            c.dblS = gp.tile([R + 2 * NS, L], BF16, tag="dblS")
            c.delta = gp.tile([P, MT_SH, L], BF16, tag="delta")
            c.u = gp.tile([P, MT_SH, L], BF16, tag="u")
            c.y2sum = gp.tile([P, MT_SH, L], BF16, tag="y2sum")
            c.state = gp.tile([P, MT_SH, NS], F32, tag="state")

            # zero the conv pads once
            for mt in range(MT_FULL):
                nc.gpsimd.memset(c.xi[:, mt, 0:3], 0.0)
                nc.gpsimd.memset(c.xi[:, mt, 3 + L:], 0.0)

            # DRAM staging for collectives (per half)
            c.bc_dram = [dp.tile([2 * NS, HL], BF16, tag=f"bc_dram{h}", name=f"bc_dram{h}")
                         for h in range(2)]
            c.up_in = [dp.tile([DIM, HL], BF16, tag=f"up_in{h}", name=f"up_in{h}")
                       for h in range(2)]
            c.up_rs = [dp.tile([P, HL], BF16, tag=f"up_rs{h}", name=f"up_rs{h}")
                       for h in range(2)]
            c.hag_out = [dp.tile([DIM, HL], BF16, tag=f"hag_out{h}", name=f"hag_out{h}")
                         for h in range(2)]

            if repeat == 1:
                build_body(c)
            else:
                with tc.For_i(0, repeat, 1):
                    build_body(c)
    nc.compile()
    return nc


def load_set_weights(c, s):
    nc = c.nc
    W = {}
    for suff in ('wxi', 'wz', 'conv', 'convb', 'xp', 'dt', 'dtb', 'A', 'D', 'op'):
        t = c.din[f'{s}_{suff}']
        wt = c.wp.tile(list(t.shape), t.dtype, tag=f"w_{suff}")
        nc.sync.dma_start(wt[:], t[:])
        W[suff] = wt
    return W


def build_body(c):
    nc = c.nc
    L = c.L
    G = c.G

    # ---- patch embedding: full h bf16 + own fp32 shard ----
    for m in range(KT):
        for j in range(L // NT):
            ps = c.pp.tile([P, NT], F32, tag="ps")
            nc.tensor.matmul(ps[:], G['patch_lhsT'][:, m, :],
                             G['x_rhs'][:, bass.ts(j, NT)], start=True, stop=True)
            nc.scalar.activation(c.hbf[:, m, bass.ts(j, NT)], ps[:], AF.Identity,
                                 bias=G['patch_b'][:, m, :])
    # ---- blocks ----
    run_block(c, 'in', pair=True)
    for i in range(DEPTH):
        run_block(c, f'l{i}', pair=False)
    run_block(c, 'out', pair=True)

    # ---- lm head: per half, right after that half's residual apply ----
    jh = c.jh
    for ph in c.pending:
        residual_apply(c, ph)
        for j in range(ph * jh, (ph + 1) * jh):
            ps = c.pp.tile([P, NT], F32, tag="ps")
            for k in range(KT):
                nc.tensor.matmul(ps[:], G['lm_lhsT'][:, k, :],
                                 c.hbf[:, k, bass.ts(j, NT)],
                                 start=(k == 0), stop=(k == KT - 1))
            ot = c.hp.tile([P, NT], F32, tag="lmout", bufs=1)
            nc.vector.tensor_copy(ot[:], ps[:])
            nc.sync.dma_start(c.out_t[:, bass.ts(j, NT)], ot[:])
    c.pending = []


def prep_half(c, W, h, rev):
    """conv + xproj + delta + u for token half h (in_proj already done)."""
    nc = c.nc
    HL, jh = c.HL, c.jh
    t0 = h * HL

    # conv (full channels) -> silu -> xc
    for mt in range(MT_FULL):
        for j in range(jh):
            ps = c.pp.tile([P, NT], F32, tag="ps")
            for k in range(KC):
                off = (6 - k) if rev else k
                nc.tensor.matmul(ps[:], W['conv'][:, k, mt, :],
                                 c.xi[:, mt, t0 + j * NT + off: t0 + j * NT + off + NT],
                                 start=(k == 0), stop=(k == KC - 1))
            nc.scalar.activation(c.xc[:, mt, t0 + j * NT: t0 + (j + 1) * NT], ps[:],
                                 AF.Silu, bias=W['convb'][:, mt, :])

    # xproj (full, local): dbl[62, HL]
    for j in range(jh):
        psf = c.pp.tile([P, NT], F32, tag="ps")
        ps = psf[0:R + 2 * NS, :]
        for kt in range(MT_FULL):
            nc.tensor.matmul(ps, W['xp'][:, kt, :],
                             c.xc[:, kt, t0 + j * NT: t0 + (j + 1) * NT],
                             start=(kt == 0), stop=(kt == MT_FULL - 1))
        nc.vector.tensor_copy(c.dblS[:, t0 + j * NT: t0 + (j + 1) * NT], ps)

    nc.sync.dma_start(c.bc_dram[h][:], c.dblS[R:R + 2 * NS, t0:t0 + HL])

    # q = sigmoid(-(dtx+dtb)); delta tile holds lnq = -softplus(dtx+dtb)
    for mt in range(MT_SH):
        for j in range(jh):
            ps = c.pp.tile([P, NT], F32, tag="ps")
            nc.tensor.matmul(ps[:], W['dt'][:, mt, :],
                             c.dblS[0:R, t0 + j * NT: t0 + (j + 1) * NT],
                             start=True, stop=True)
            nc.scalar.activation(c.u[:, mt, t0 + j * NT: t0 + (j + 1) * NT],
                                 ps[:], AF.Sigmoid, bias=W['dtb'][:, mt, :],
                                 scale=-1.0)
    for mt in range(MT_SH):
        nc.scalar.activation(c.delta[:, mt, t0:t0 + HL], c.u[:, mt, t0:t0 + HL],
                             AF.Ln)
    # u = (-lnq) * xc_shard = softplus * xc
    for mt in range(MT_SH):
        nc.vector.scalar_tensor_tensor(c.u[:, mt, t0:t0 + HL],
                                       c.delta[:, mt, t0:t0 + HL], -1.0,
                                       c.xc[:, mt, t0:t0 + HL],
                                       op0=OP.mult, op1=OP.mult)


def scan_half(c, W, h, rev, first_half, di):
    """Selective scan + gating for token half h of direction di."""
    nc = c.nc
    HL, jh = c.HL, c.jh
    t0 = h * HL

    yps = [c.yp.tile([P, HL], F32, tag=f"yacc{mt}", name=f"yacc{mt}")
           for mt in range(MT_SH)]
    for n in range(NS):
        bcrep = c.sp.tile([P, 2, HL], BF16, tag="bcrep", bufs=4)
        nc.sync.dma_start(bcrep[:, None, :, :],
                          c.bc_dram[h][2 * n:2 * n + 2, :].partition_broadcast(P))
        brep = bcrep[:, 0, :]
        crep = bcrep[:, 1, :]
        bgeng = nc.gpsimd if (n % 3 == 1) else nc.vector
        hgeng = bgeng
        for mt in range(MT_SH):
            ag = c.sp.tile([P, HL], BF16, tag=f"ag{mt}", bufs=2)
            nc.scalar.activation(ag[:], c.delta[:, mt, t0:t0 + HL], AF.Exp,
                                 scale=W['A'][:, mt, n:n + 1])

            bg = c.sp.tile([P, HL], BF16, tag=f"bg{mt}", bufs=3)
            bgeng.tensor_mul(bg[:], c.u[:, mt, t0:t0 + HL], brep)

            hg = c.sp.tile([P, HL], BF16, tag=f"hg{mt}", bufs=3)
            init = 0.0 if first_half else c.state[:, mt, n:n + 1]
            eng = nc.vector
            if rev:
                eng.tensor_tensor_scan(hg[:, ::-1], ag[:, ::-1], bg[:, ::-1],
                                       init, OP.mult, OP.add)
                if first_half:
                    nc.vector.tensor_copy(c.state[:, mt, n:n + 1], hg[:, 0:1])
            else:
                eng.tensor_tensor_scan(hg[:], ag[:], bg[:], init, OP.mult, OP.add)
                if first_half:
                    nc.vector.tensor_copy(c.state[:, mt, n:n + 1], hg[:, HL - 1:HL])

            hgc = c.sp.tile([P, HL], BF16, tag=f"bg{mt}", bufs=3, name=f"hgc{mt}")
            hgeng.tensor_mul(hgc[:], hg[:], crep)
            for ch in range(jh):
                nc.tensor.matmul(yps[mt][:, bass.ts(ch, NT)], c.ident[:],
                                 hgc[:, bass.ts(ch, NT)],
                                 start=(n == 0), stop=(n == NS - 1))

    # gating: y2 = yacc + D*xc ; y2s = y2 * sz (accumulate over directions)
    for ch in range(jh):
        for mt in range(MT_SH):
            tc0 = t0 + ch * NT
            y2 = c.hp.tile([P, NT], BF16, tag="y2", bufs=2)
            nc.vector.scalar_tensor_tensor(y2[:], c.xc[:, mt, tc0:tc0 + NT],
                                           W['D'][:, mt, :],
                                           yps[mt][:, bass.ts(ch, NT)],
                                           op0=OP.mult, op1=OP.add)
            if di == 0:
                nc.vector.tensor_mul(c.y2sum[:, mt, tc0:tc0 + NT], y2[:],
                                     c.sz[:, mt, tc0:tc0 + NT])
            else:
                y3 = c.hp.tile([P, NT], BF16, tag="y3", bufs=2)
                nc.gpsimd.tensor_mul(y3[:], y2[:], c.sz[:, mt, tc0:tc0 + NT])
                nc.vector.tensor_add(c.y2sum[:, mt, tc0:tc0 + NT],
                                     c.y2sum[:, mt, tc0:tc0 + NT], y3[:])


def residual_update(c, W, h):
    """out_proj partials for half h -> RS -> AG of the delta (no staging)."""
    nc = c.nc
    HL, jh = c.HL, c.jh
    t0 = h * HL
    for m in range(KT):
        st = c.hp.tile([P, HL], BF16, tag="opst", bufs=2)
        for j in range(jh):
            ps = c.pp.tile([P, NT], F32, tag="ps")
            for kt in range(MT_SH):
                nc.tensor.matmul(ps[:], W['op'][:, kt, m, :],
                                 c.y2sum[:, kt, t0 + j * NT: t0 + (j + 1) * NT],
                                 start=(kt == 0), stop=(kt == MT_SH - 1))
            nc.scalar.activation(st[:, bass.ts(j, NT)], ps[:], AF.Copy)
        nc.sync.dma_start(c.up_in[h][m * P:(m + 1) * P, :], st[:])

    nc.gpsimd.collective_compute(
        "ReduceScatter", OP.add, replica_groups=c.groups,
        ins=[c.up_in[h][:].opt()], outs=[c.up_rs[h][:].opt()])
    nc.gpsimd.collective_compute(
        "AllGather", OP.bypass, replica_groups=c.groups,
        ins=[c.up_rs[h][:].opt()], outs=[c.hag_out[h][:].opt()])


def residual_apply(c, h):
    """Load the gathered residual delta and add it into hbf."""
    nc = c.nc
    HL = c.HL
    t0 = h * HL
    for k in range(KT):
        upf = c.sp.tile([P, HL], BF16, tag="upf", bufs=2, name=f"upf{k}")
        nc.sync.dma_start(upf[:], c.hag_out[h][k * P:(k + 1) * P, :])
        nc.vector.tensor_add(c.hbf[:, k, t0:t0 + HL], c.hbf[:, k, t0:t0 + HL],
                             upf[:])


def dirs_of(pair):
    return [(False, 0), (True, 1)] if pair else [(False, 0)]


def in_proj_half(c, W, h):
    nc = c.nc
    HL, jh = c.HL, c.jh
    for j in range(h * jh, (h + 1) * jh):
        for mt in range(MT_FULL):
            ps = c.pp.tile([P, NT], F32, tag="ps")
            for k in range(KT):
                nc.tensor.matmul(ps[:], W['wxi'][:, k, mt, :],
                                 c.hbf[:, k, bass.ts(j, NT)],
                                 start=(k == 0), stop=(k == KT - 1))
            dst = c.xi[:, mt, 3 + j * NT: 3 + (j + 1) * NT]
            nc.scalar.activation(dst, ps[:], AF.Copy)
        for mt in range(MT_SH):
            ps = c.pp.tile([P, NT], F32, tag="ps")
            for k in range(KT):
                nc.tensor.matmul(ps[:], W['wz'][:, k, mt, :],
                                 c.hbf[:, k, bass.ts(j, NT)],
                                 start=(k == 0), stop=(k == KT - 1))
            nc.scalar.activation(c.sz[:, mt, bass.ts(j, NT)], ps[:], AF.Silu)


def run_block(c, s, pair):
    nc = c.nc
    L = c.L
    for ph in c.pending:
        residual_apply(c, ph)
    c.pending = []
    W = load_set_weights(c, s)

    for rev, di in dirs_of(pair):
        if not rev:
            for h in (0, 1):
                in_proj_half(c, W, h)
                prep_half(c, W, h, rev)
                scan_half(c, W, h, rev, h == 0, di)
                if not pair:
                    residual_update(c, W, h)
            if not pair:
                c.pending = [0, 1]
        else:
            # reverse direction of a pair: xi already complete; h1 first
            prep_half(c, W, 1, rev)
            scan_half(c, W, 1, rev, True, di)
            residual_update(c, W, 1)
            prep_half(c, W, 0, rev)
            scan_half(c, W, 0, rev, False, di)
            residual_update(c, W, 0)
            c.pending = [1, 0]


_KERNEL_CACHE = {}


def get_kernel(L, repeat=1):
    key = (L, repeat)
    if key not in _KERNEL_CACHE:
        _KERNEL_CACHE[key] = build_kernel(L, repeat)
    return _KERNEL_CACHE[key]


def kernel(**inputs):
    L = int(np.asarray(inputs['x']).shape[1])
    nc = get_kernel(L)
    in_maps = [prep_core_inputs(cc, inputs, L) for cc in range(NCORES)]
    res = run_bass_kernel_spmd(nc, in_maps, list(range(NCORES)))
    outs = [np.asarray(res.results[cc]['out'], np.float32) for cc in range(NCORES)]
    full = []
    for srow in range(B):
        sm = np.concatenate(outs[srow * TPG:(srow + 1) * TPG], axis=0)  # (472, L)
        full.append(sm.T)
    return np.ascontiguousarray(np.stack(full, axis=0))
